# revision 12
# baseline (speedup 1.0000x reference)
"""Charge-equilibration kernel for Trainium2 (8 NeuronCores, SPMD) — v2.

Problem: 1024 molecules x 128 atoms. Per molecule build the erf-screened
Coulomb matrix A, solve the augmented system via Schur complement, return
partial charges [131072] f32.

v2 algorithm (per core: 128 molecules, data-parallel across cores):
  - Jacobi-scaled system At = D_s A D_s (unit diagonal), M = At/S0.
  - Host packs scaled build tensors so the device computes
      d~^2 = d^2 * S0^2/(s_i s_j)^2  (PE, paired-molecule f32r matmuls)
      dt  = sqrt(d~^2)               (ACT)
      rds = 1/dt = s_i s_j/(S0 d)    (DVE reciprocal)
      x   = dt * rgs                 (Pool; rgs = s_i s_j/(S0*sqrt(2)gam),
                                      f32 host pack, diag 0 -> Gf diag 0)
      ev  = erf(x)                   (ACT)
      Gf  = ev * rds                 (Pool; f32 == offdiag(At)/S0)
      g   = bf16(Gf)
  - Cubic Chebyshev seed on [a,b]:
      E0 = a0 I + a1 G + a2 G^2 + a3 G^3,  X0 = b0 I + b1 G + b2 G^2
    built via PE PSUM accumulation (G2k = bf16(a3*G^2) via ACT scale-copy).
  - K-1 product-form NS squarings E_{j+1} = E_j^2 (bf16), chain applies
    to thin rhs ([128, 2/molecule]) + R refinements against exact f32 Gf.
  - Schur: lam = (Q + sum v)/(1 - sum u), q = -(v + lam*u).

Emission is software-pipelined over cohorts of 4 superquads (8 molecules
each, [128,1024] tiles) with phase-batched ACT table usage.
"""

import os
import numpy as np

import concourse.bass as bass
import concourse.bacc as bacc
import concourse.tile as tile
import concourse.mybir as mybir
from concourse.bass_utils import run_bass_kernel_spmd
from concourse.masks import make_identity

dt = mybir.dt
AF = mybir.ActivationFunctionType
ALU = mybir.AluOpType

N_CORES = 8
B_MOL = 1024
N_ATOM = 128
MPC = B_MOL // N_CORES          # molecules per core = 128
SQ = 8                          # molecules per superquad
NSQ = MPC // SQ                 # 16 superquads
CSQ = 4                         # superquads per cohort
NCOH = NSQ // CSQ               # 4 cohorts
NM = CSQ * SQ                   # molecules per cohort = 32
W = SQ * N_ATOM                 # superquad tile width = 1024

S0 = float(os.environ.get("KE_S0", "32.5"))
CH_A = float(os.environ.get("KE_A", "0.0425"))
CH_B = float(os.environ.get("KE_B", "34.5"))
K_NS = int(os.environ.get("KE_K", "6"))     # chain length (K-1 squarings)
N_REF = int(os.environ.get("KE_R", "3"))
OFF = int(os.environ.get("KE_OFF", "12"))   # cohort pipeline offset (ticks)
EPS_D2 = 1.0e-4
L_DIAG = 1.0e10
SQRT_L = float(np.sqrt(L_DIAG))
C0 = 1.0 / S0

# engine knobs: a=ACT, v=DVE, p=Pool
E_SE = os.environ.get("KE_SE", "a")             # E0 evacuation
ECOPY = os.environ.get("KE_ECOPY", "vavav")     # squaring evacs (K_NS-1)
E_GBF = os.environ.get("KE_GBF", "p")           # g = bf16(Gf)
E_CV0 = os.environ.get("KE_CV0", "v")           # cv0 thin evac
BUILD_F32R = os.environ.get("KE_BF32R", "0") == "1"


def _cheb_seed(a, b, deg, S0v):
    import numpy.polynomial.chebyshev as C
    import numpy.polynomial.polynomial as P
    from math import comb
    cheb = np.zeros(deg + 1)
    cheb[deg] = 1.0
    tpoly = C.cheb2poly(cheb)
    u0 = (b + a) / (b - a)
    u1 = -2.0 / (b - a)
    t_lam = np.zeros(deg + 1)
    for k in range(deg + 1):
        if tpoly[k] == 0.0:
            continue
        binom = P.polypow([u0, u1], k) if k > 0 else np.array([1.0])
        t_lam[:len(binom)] += tpoly[k] * binom
    t0 = np.polynomial.chebyshev.chebval(u0, cheb)
    e_lam = t_lam / t0
    x_lam = -e_lam[1:]
    alpha = np.zeros(deg + 1)
    beta = np.zeros(deg)
    c0 = 1.0 / S0v
    for k in range(deg + 1):
        ck = e_lam[k] * S0v ** k
        for j in range(k + 1):
            alpha[j] += ck * comb(k, j) * c0 ** (k - j)
    for k in range(deg):
        ck = x_lam[k] * S0v ** k
        for j in range(k + 1):
            beta[j] += ck * comb(k, j) * c0 ** (k - j)
    beta *= S0v
    return alpha, beta


ALPHA, BETA = _cheb_seed(CH_A, CH_B, 3, S0)
SCL3 = float(ALPHA[3])          # exact f32 scale used in G2k ACT copy
RB1 = float(BETA[1])            # rhs1 scale
RB2 = float(BETA[2] / SCL3)     # rhs2 scale (G2k-term)


def _bf_split(v):
    hi = float(np.float32(v).astype(np.float16).astype(np.float32))
    lo = float(np.float32(v - hi))
    return hi, lo


A1_HI, A1_LO = _bf_split(ALPHA[1])
R23_HI, R23_LO = _bf_split(ALPHA[2] / SCL3)

_CACHE = {}


def _build_bass():
    nc = bacc.Bacc()
    f32 = dt.float32
    bf = dt.float16
    bfL = dt.bfloat16

    NPAIR = MPC // 2
    lhs = nc.declare_dram_parameter("lhsr_pack", [10, NPAIR * N_ATOM], f32,
                                    isOutput=False)
    rhs = nc.declare_dram_parameter("rhsr_pack", [10, NPAIR * 2 * N_ATOM],
                                    f32, isOutput=False)
    rgs = nc.declare_dram_parameter("rgs_pack", [N_ATOM, MPC * N_ATOM], f32,
                                    isOutput=False)
    btp = nc.declare_dram_parameter("bt_pack", [N_ATOM, 2 * MPC], f32,
                                    isOutput=False)
    bt1p = nc.declare_dram_parameter("bt1_pack", [N_ATOM, 2 * MPC], bf,
                                     isOutput=False)
    bt2p = nc.declare_dram_parameter("bt2_pack", [N_ATOM, 2 * MPC], bf,
                                     isOutput=False)
    shS = nc.declare_dram_parameter("shS_pack", [N_ATOM, 2 * MPC], f32,
                                    isOutput=False)
    qpk = nc.declare_dram_parameter("q_pack", [1, MPC], f32, isOutput=False)
    out = nc.declare_dram_parameter("out", [N_ATOM, MPC], f32, isOutput=True)
    DBG = os.environ.get("KE_DBG", "") == "1"
    dbg = {}
    if DBG:
        for nm, w_ in [("d_dt", W), ("d_rds", W), ("d_ev", W), ("d_gf", W),
                       ("d_g2k", W), ("d_se", W), ("d_e1", W), ("d_w0", 64),
                       ("d_w1", 64)]:
            dbg[nm] = nc.declare_dram_parameter(nm, [N_ATOM, w_], f32,
                                                isOutput=True)

    def cp(eng, dst, src):
        if eng == "a":
            nc.scalar.copy(dst, src)
        elif eng == "v":
            nc.vector.tensor_copy(dst, src)
        else:
            nc.gpsimd.tensor_copy(dst, src)

    from contextlib import ExitStack

    with tile.TileContext(nc) as tc:
        with ExitStack() as es:
            def pool(name, bufs, space=None):
                kw = {"space": space} if space else {}
                return es.enter_context(
                    tc.tile_pool(name=name, bufs=bufs, **kw))

            const = pool("const", 1)
            core_in = pool("core_in", 1)
            DP = int(os.environ.get("KE_DP", "3"))
            lhs_in = pool("lhs_in", DP)
            rhs_in = pool("rhs_in", DP)
            rgs_in = pool("rgs_in", DP + 1)
            dtp = pool("dtp", DP)
            rdsp = pool("rdsp", DP)
            xp = pool("xp", DP)
            gfp = pool("gfp", CSQ + 2)
            gp = pool("gp", CSQ + 2)
            g2kp = pool("g2kp", CSQ + 2)
            sep = pool("sep", K_NS * CSQ + 4)
            cvp = pool("cvp", 4)
            wp = pool("wp", 5)
            fint = pool("fint", 8)
            lamp = pool("lamp", 8)
            p_wide = pool("p_wide", 3, "PSUM")
            p_thin = pool("p_thin", 2, "PSUM")

            # ---- constants ----
            identI = const.tile([128, 128], bf)
            make_identity(nc, identI[:])          # plain identity (for k1blk)
            k2I = const.tile([128, 128], bf)
            nc.gpsimd.memset(k2I[:], A1_HI)
            nc.gpsimd.affine_select(
                out=k2I[:], in_=k2I[:], compare_op=ALU.is_equal,
                fill=0.0, base=0, pattern=[[-1, 128]], channel_multiplier=1)
            r23I = const.tile([128, 128], bf)
            nc.gpsimd.memset(r23I[:], R23_HI)
            nc.gpsimd.affine_select(
                out=r23I[:], in_=r23I[:], compare_op=ALU.is_equal,
                fill=0.0, base=0, pattern=[[-1, 128]], channel_multiplier=1)
            k2lo = const.tile([128, 128], bf)
            nc.gpsimd.memset(k2lo[:], A1_LO)
            nc.gpsimd.affine_select(
                out=k2lo[:], in_=k2lo[:], compare_op=ALU.is_equal,
                fill=0.0, base=0, pattern=[[-1, 128]], channel_multiplier=1)
            r23lo = const.tile([128, 128], bf)
            nc.gpsimd.memset(r23lo[:], R23_LO)
            nc.gpsimd.affine_select(
                out=r23lo[:], in_=r23lo[:], compare_op=ALU.is_equal,
                fill=0.0, base=0, pattern=[[-1, 128]], channel_multiplier=1)
            b0I = const.tile([128, 128], bf)
            nc.gpsimd.memset(b0I[:], float(BETA[0]))
            nc.gpsimd.affine_select(
                out=b0I[:], in_=b0I[:], compare_op=ALU.is_equal,
                fill=0.0, base=0, pattern=[[-1, 128]], channel_multiplier=1)
            identL = const.tile([128, 128], bfL)
            nc.gpsimd.memset(identL[:], SQRT_L)
            nc.gpsimd.affine_select(
                out=identL[:], in_=identL[:], compare_op=ALU.is_equal,
                fill=0.0, base=0, pattern=[[-1, 128]], channel_multiplier=1)
            k1blk = const.tile([128, W], bf)
            nc.gpsimd.memset(k1blk[:], float(ALPHA[0]))
            nc.gpsimd.affine_select(
                out=k1blk[:], in_=k1blk[:], compare_op=ALU.is_equal,
                fill=0.0, base=0, pattern=[[0, SQ], [-1, 128]],
                channel_multiplier=1)
            ones_col = const.tile([128, 1], f32)
            nc.gpsimd.memset(ones_col[:], 1.0)

            # ---- whole-core small inputs ----
            bt_all = core_in.tile([N_ATOM, 2 * MPC], f32)
            nc.sync.dma_start(bt_all[:], btp[:])
            bt1_all = core_in.tile([N_ATOM, 2 * MPC], bf)
            nc.sync.dma_start(bt1_all[:], bt1p[:])
            bt2_all = core_in.tile([N_ATOM, 2 * MPC], bf)
            nc.sync.dma_start(bt2_all[:], bt2p[:])
            shS_all = core_in.tile([N_ATOM, 2 * MPC], f32)
            nc.sync.dma_start(shS_all[:], shS[:])
            q_all = core_in.tile([1, MPC], f32)
            nc.sync.dma_start(q_all[:], qpk[:])
            btb_all = core_in.tile([N_ATOM, 2 * MPC], bf)
            nc.vector.tensor_copy(btb_all[:], bt_all[:])

            # ---- phases (cohort state dicts) ----
            def ph_build(st, c):
                st["pd"] = {}
                st["dt"] = {}
                for si in range(CSQ):
                    sq = c * CSQ + si
                    lh = lhs_in.tile([10, 4 * 128], f32, tag="lh")
                    nc.sync.dma_start(
                        lh[:], lhs[:, sq * 4 * 128:(sq + 1) * 4 * 128])
                    rh = rhs_in.tile([10, 4 * 256], f32, tag="rh")
                    nc.sync.dma_start(
                        rh[:], rhs[:, sq * 4 * 256:(sq + 1) * 4 * 256])
                    rg = rgs_in.tile([128, W], f32, tag="rg")
                    nc.sync.dma_start(
                        rg[:], rgs[:, sq * W:(sq + 1) * W])
                    st.setdefault("rg", {})[si] = rg
                    pd = p_wide.tile([128, W], f32, tag="w")
                    for p in range(4):      # pairs of molecules
                        lsl = lh[:, p * 128:(p + 1) * 128]
                        rsl = rh[:, p * 256:(p + 1) * 256]
                        if BUILD_F32R:
                            lsl = lsl.bitcast(dt.float32r)
                            rsl = rsl.bitcast(dt.float32r)
                        ps = slice(p * 256, (p + 1) * 256)
                        nc.tensor.matmul(pd[:, ps], lsl, rsl,
                                         start=True, stop=False)
                        nc.tensor.matmul(
                            pd[:, p * 256:p * 256 + 128], identL[:],
                            identL[:], start=False, stop=False)
                        nc.tensor.matmul(
                            pd[:, p * 256 + 128:(p + 1) * 256], identL[:],
                            identL[:], start=False, stop=True)
                    st["pd"][si] = pd

            def ph_sqrt(st, c):
                for si in range(CSQ):
                    pd = st["pd"].pop(si)
                    dts = dtp.tile([128, W], f32, tag="dt")
                    nc.scalar.sqrt(dts[:], pd[:])
                    st["dt"][si] = dts
                    if DBG and c == 0 and si == 3:
                        nc.sync.dma_start(dbg["d_dt"][:], dts[:])

            def ph_rx(st, c):
                st["rds"] = {}
                st["x"] = {}
                for si in range(CSQ):
                    dts = st["dt"][si]
                    rds = rdsp.tile([128, W], f32, tag="rds")
                    nc.vector.reciprocal(rds[:], dts[:])
                    st["rds"][si] = rds
                    x = xp.tile([128, W], f32, tag="x")
                    nc.gpsimd.tensor_mul(x[:], dts[:], st["rg"][si][:])
                    st["x"][si] = x
                st["dt"].clear()

            def ph_erf(st, c):
                for si in range(CSQ):
                    x = st["x"][si]
                    nc.scalar.activation(x[:], x[:], AF.Erf)   # in-place

            def ph_gf(st, c):
                st["gf"] = {}
                st["g"] = {}
                for si in range(CSQ):
                    x = st["x"].pop(si)
                    rds = st["rds"].pop(si)
                    gf = gfp.tile([128, W], f32, tag="gf")
                    nc.gpsimd.tensor_mul(gf[:], x[:], rds[:])
                    st["gf"][si] = gf
                    if DBG and c == 0 and si == 3:
                        nc.sync.dma_start(dbg["d_rds"][:], rds[:])
                        nc.sync.dma_start(dbg["d_ev"][:], x[:])
                        nc.sync.dma_start(dbg["d_gf"][:], gf[:])
                    g = gp.tile([128, W], bf, tag="g")
                    cp(E_GBF, g[:], gf[:])
                    st["g"][si] = g
                st["rg"].clear()

            def ph_warm(st, c):
                # sub-loops per stage so PE/ACT pipeline across superquads
                st["es"] = {}
                st["g2k"] = {}
                pbs, pb2s = {}, {}
                for si in range(CSQ):
                    g = st["g"][si]
                    pb = p_wide.tile([128, W], f32, tag="w")
                    for m in range(SQ):
                        sl = slice(m * 128, (m + 1) * 128)
                        nc.tensor.matmul(pb[:, sl], g[:, sl], g[:, sl],
                                         start=True, stop=True)
                    pbs[si] = pb
                    g2k = g2kp.tile([128, W], bf, tag="g2k")
                    nc.scalar.activation(g2k[:], pb[:], AF.Copy, scale=SCL3)
                    st["g2k"][si] = g2k
                for si in range(CSQ):
                    g = st["g"][si]
                    g2k = st["g2k"][si]
                    pb2 = p_wide.tile([128, W], f32, tag="w")
                    for h in range(2):     # two 512-col halves (PSUM banks)
                        hs = slice(h * 512, (h + 1) * 512)
                        nc.tensor.matmul(pb2[:, hs], k2I[:], g[:, hs],
                                         start=True, stop=False)
                        nc.tensor.matmul(pb2[:, hs], k2lo[:], g[:, hs],
                                         start=False, stop=False)
                        nc.tensor.matmul(pb2[:, hs], r23I[:], g2k[:, hs],
                                         start=False, stop=False)
                        nc.tensor.matmul(pb2[:, hs], r23lo[:], g2k[:, hs],
                                         start=False, stop=False)
                        for m in range(4 * h, 4 * h + 4):
                            sl = slice(m * 128, (m + 1) * 128)
                            nc.tensor.matmul(pb2[:, sl], g2k[:, sl],
                                             g[:, sl], start=False,
                                             stop=False)
                        nc.tensor.matmul(pb2[:, hs], identI[:],
                                         k1blk[:, hs], start=False,
                                         stop=True)
                    pb2s[si] = pb2
                    se = sep.tile([128, W], bf, tag="se")
                    cp(E_SE, se[:], pb2[:])
                    st["es"][si] = [se]
                    if DBG and c == 0 and si == 3:
                        tmp = xp.tile([128, W], f32, tag="x")
                        nc.vector.tensor_copy(tmp[:], st["g2k"][si][:])
                        nc.sync.dma_start(dbg["d_g2k"][:], tmp[:])
                        tmp2 = xp.tile([128, W], f32, tag="x")
                        nc.vector.tensor_copy(tmp2[:], se[:])
                        nc.sync.dma_start(dbg["d_se"][:], tmp2[:])

            def ph_ns(st, c, k):
                for si in range(CSQ):
                    se = st["es"][si][-1]
                    eb = p_wide.tile([128, W], f32, tag="w")
                    for m in range(SQ):
                        sl = slice(m * 128, (m + 1) * 128)
                        nc.tensor.matmul(eb[:, sl], se[:, sl], se[:, sl],
                                         start=True, stop=True)
                    se2 = sep.tile([128, W], bf, tag="se")
                    cp(ECOPY[k], se2[:], eb[:])
                    st["es"][si].append(se2)
                    if DBG and c == 0 and si == 3 and k == 0:
                        tmp = xp.tile([128, W], f32, tag="x")
                        nc.vector.tensor_copy(tmp[:], se2[:])
                        nc.sync.dma_start(dbg["d_e1"][:], tmp[:])

            def emit_apply(st, c, rhs_bf, rhs1, rhs2, w_prev):
                """w = (w_prev +) chain(rhs): X0 = b0 I + b1 G + (b2/a3) G2k,
                then K stages of (I+E_j). rhs1 = RB1*rhs, rhs2 = RB2*rhs."""
                gb = p_thin.tile([128, 2 * NM], f32, tag="t")
                for mi in range(NM):
                    si, m = mi // SQ, mi % SQ
                    sl = slice(m * 128, (m + 1) * 128)
                    ts = slice(2 * mi, 2 * mi + 2)
                    nc.tensor.matmul(gb[:, ts], b0I[:], rhs_bf[:, ts],
                                     start=True, stop=False)
                    nc.tensor.matmul(gb[:, ts], st["g"][si][:, sl],
                                     rhs1[:, ts], start=False, stop=False)
                    nc.tensor.matmul(gb[:, ts], st["g2k"][si][:, sl],
                                     rhs2[:, ts], start=False, stop=True)
                cv = cvp.tile([128, 2 * NM], bf, tag="cv")
                cp(E_CV0, cv[:], gb[:])
                for j in range(K_NS):
                    cb = p_thin.tile([128, 2 * NM], f32, tag="t")
                    for mi in range(NM):
                        si, m = mi // SQ, mi % SQ
                        sl = slice(m * 128, (m + 1) * 128)
                        nc.tensor.matmul(
                            cb[:, 2 * mi:2 * mi + 2],
                            st["es"][si][j][:, sl],
                            cv[:, 2 * mi:2 * mi + 2],
                            start=(mi == 0), stop=(mi == NM - 1))
                    if j < K_NS - 1:
                        cv2 = cvp.tile([128, 2 * NM], bf, tag="cv")
                        nc.vector.tensor_add(cv2[:], cv[:], cb[:])
                        cv = cv2
                    else:
                        w = wp.tile([128, 2 * NM], f32, tag="w")
                        if w_prev is None:
                            nc.vector.tensor_add(w[:], cv[:], cb[:])
                        else:
                            cv3 = fint.tile([128, 2 * NM], f32, tag="cv3")
                            nc.vector.tensor_add(cv3[:], cv[:], cb[:])
                            nc.vector.tensor_add(w[:], w_prev[:], cv3[:])
                return w

            def ph_fa(st, c):
                csl = slice(c * 2 * NM, (c + 1) * 2 * NM)
                st["w"] = emit_apply(st, c, btb_all[:, csl],
                                     bt1_all[:, csl], bt2_all[:, csl], None)
                if DBG and c == 0:
                    nc.sync.dma_start(dbg["d_w0"][:], st["w"][:])

            def ph_fr(st, c):
                csl = slice(c * 2 * NM, (c + 1) * 2 * NM)
                w = st["w"]
                t2 = fint.tile([128, 2 * NM], f32, tag="t2")
                nc.vector.scalar_tensor_tensor(
                    out=t2[:], in0=w[:], scalar=-C0, in1=bt_all[:, csl],
                    op0=ALU.mult, op1=ALU.add)
                pp = p_thin.tile([128, 2 * NM], f32, tag="t")
                for mi in range(NM):
                    si, m = mi // SQ, mi % SQ
                    sl = slice(m * 128, (m + 1) * 128)
                    nc.tensor.matmul(pp[:, 2 * mi:2 * mi + 2],
                                     st["gf"][si][:, sl],
                                     w[:, 2 * mi:2 * mi + 2],
                                     start=(mi == 0), stop=(mi == NM - 1))
                rt = fint.tile([128, 2 * NM], bf, tag="rt")
                nc.vector.scalar_tensor_tensor(
                    out=rt[:], in0=pp[:], scalar=-1.0, in1=t2[:],
                    op0=ALU.mult, op1=ALU.add)
                rt1 = fint.tile([128, 2 * NM], bf, tag="rt1")
                nc.vector.tensor_scalar_mul(rt1[:], rt[:], RB1)
                rt2 = fint.tile([128, 2 * NM], bf, tag="rt2")
                nc.vector.tensor_scalar_mul(rt2[:], rt[:], RB2)
                st["w"] = emit_apply(st, c, rt, rt1, rt2, w)
                if DBG and c == 0:
                    nc.sync.dma_start(dbg["d_w1"][:], st["w"][:])

            def ph_fs(st, c):
                csl = slice(c * 2 * NM, (c + 1) * 2 * NM)
                ws = lamp.tile([128, 2 * NM], f32, tag="ws")
                nc.vector.tensor_mul(ws[:], st["w"][:], shS_all[:, csl])
                sums = p_thin.tile([1, 2 * NM], f32, tag="t")
                nc.tensor.matmul(sums[:], ones_col[:], ws[:])
                num = lamp.tile([1, NM], f32, tag="num")
                nc.vector.tensor_add(
                    num[:], sums[0:1, 0:2 * NM:2],
                    q_all[:, c * NM:(c + 1) * NM])
                den = lamp.tile([1, NM], f32, tag="den")
                nc.vector.tensor_scalar_add(den[:], sums[0:1, 1:2 * NM:2],
                                            -1.0)
                rden = lamp.tile([1, NM], f32, tag="rden")
                nc.vector.reciprocal(rden[:], den[:])
                lamneg = lamp.tile([1, NM], f32, tag="lamneg")
                nc.vector.tensor_mul(lamneg[:], num[:], rden[:])
                lamb = lamp.tile([128, NM], f32, tag="lamb")
                nc.gpsimd.partition_broadcast(lamb[:], lamneg[:])
                t1 = lamp.tile([128, NM], f32, tag="t1")
                nc.vector.tensor_mul(t1[:], ws[:, 1:2 * NM:2], lamb[:])
                qc = lamp.tile([128, NM], f32, tag="qc")
                nc.vector.tensor_sub(qc[:], t1[:], ws[:, 0:2 * NM:2])
                nc.sync.dma_start(out[:, c * NM:(c + 1) * NM], qc[:])
                st["es"].clear()
                st["gf"].clear()
                st["g"].clear()
                st["g2k"].clear()

            # phase table
            def emit_phase(st, c, ph):
                if ph == 0:
                    ph_build(st, c)
                elif ph == 1:
                    ph_sqrt(st, c)
                elif ph == 2:
                    ph_rx(st, c)
                elif ph == 3:
                    ph_erf(st, c)
                elif ph == 4:
                    ph_gf(st, c)
                elif ph == 5:
                    ph_warm(st, c)
                elif ph < 5 + K_NS:
                    ph_ns(st, c, ph - 6)
                elif ph == 5 + K_NS:
                    ph_fa(st, c)
                elif ph < 6 + K_NS + N_REF:
                    ph_fr(st, c)
                else:
                    ph_fs(st, c)

            NPH = 7 + K_NS + N_REF
            states = [dict() for _ in range(NCOH)]
            total = OFF * (NCOH - 1) + NPH
            for t in range(total):
                for c in range(NCOH):
                    ph = t - OFF * c
                    if 0 <= ph < NPH:
                        emit_phase(states[c], c, ph)

    nc.compile()
    return nc


def _host_pack(eneg, positions, node_attrs, hardness, total_charge,
               atomic_numbers):
    """Precompute per-atom quantities and pack per-core DRAM tensors."""
    f32 = np.float32
    pos = np.ascontiguousarray(positions, dtype=f32).reshape(B_MOL, N_ATOM, 3)
    Z = np.asarray(atomic_numbers).astype(np.int64).reshape(B_MOL, N_ATOM)
    na = np.asarray(node_attrs, dtype=f32).reshape(B_MOL, N_ATOM, -1)
    hard = np.asarray(hardness, dtype=f32)
    e = np.asarray(eneg, dtype=f32).reshape(B_MOL, N_ATOM)
    Q = np.asarray(total_charge, dtype=f32).reshape(B_MOL)

    cov = (0.3 + 0.02 * np.arange(100)).astype(f32)
    r = cov[Z]                                   # [B, n]
    sig = (r * r).astype(f32)
    n2 = (pos * pos).sum(axis=2, dtype=f32).astype(f32)
    aidx = na.argmax(axis=2)
    dv = (hard[aidx] + f32(1.0) / (np.sqrt(np.pi).astype(f32) * r)).astype(f32)
    sh = (f32(1.0) / np.sqrt(dv)).astype(f32)    # s = 1/sqrt(diag A)

    def to_fp16(x):
        return np.ascontiguousarray(
            np.asarray(x, dtype=np.float32).astype(np.float16))

    mpc = MPC
    npair = mpc // 2
    in_maps = []
    for c in range(N_CORES):
        sl = slice(c * mpc, (c + 1) * mpc)
        p = pos[sl]          # [mpc, 128, 3]
        nn2 = n2[sl]
        sgl = sig[sl]
        shl = sh[sl]         # [mpc, 128]
        el = e[sl]

        F = (f32(S0) / (shl * shl)).astype(f32)       # S0/s^2  [mpc, n]
        # per-molecule scaled lhs rows [5, n] and rhs rows [5, n]
        lhs5 = np.stack([-2.0 * p[:, :, 0] * F, -2.0 * p[:, :, 1] * F,
                         -2.0 * p[:, :, 2] * F, (nn2 + EPS_D2) * F, F],
                        axis=1).astype(f32)            # [mpc, 5, n]
        rhs5 = np.stack([p[:, :, 0] * F, p[:, :, 1] * F, p[:, :, 2] * F,
                         F, nn2 * F], axis=1).astype(f32)

        lhsp = np.zeros((10, npair, N_ATOM), dtype=f32)
        lhsp[0:5] = lhs5[0::2].transpose(1, 0, 2)
        lhsp[5:10] = lhs5[1::2].transpose(1, 0, 2)
        rhsp = np.zeros((10, npair, 2 * N_ATOM), dtype=f32)
        rhsp[0:5, :, :N_ATOM] = rhs5[0::2].transpose(1, 0, 2)
        rhsp[5:10, :, N_ATOM:] = rhs5[1::2].transpose(1, 0, 2)

        # rgs = s_i s_j / (S0 * sqrt(2 sig_i + 2 sig_j)), diag 0
        gam2 = 2.0 * (sgl[:, :, None] + sgl[:, None, :])
        rgsp = (np.einsum("mi,mj->mij", shl, shl)
                / (f32(S0) * np.sqrt(gam2))).astype(f32)
        ii = np.arange(N_ATOM)
        rgsp[:, ii, ii] = 0.0
        rgsp = np.ascontiguousarray(
            rgsp.transpose(1, 0, 2).reshape(N_ATOM, mpc * N_ATOM))

        btpk = np.empty((N_ATOM, 2 * mpc), dtype=f32)
        btpk[:, 0::2] = (el * shl / f32(S0)).T
        btpk[:, 1::2] = (shl / f32(S0)).T
        shSp = np.empty((N_ATOM, 2 * mpc), dtype=f32)
        shSp[:, 0::2] = shl.T
        shSp[:, 1::2] = shl.T
        qp = np.ascontiguousarray(Q[sl]).reshape(1, mpc)
        in_maps.append({
            "lhsr_pack": np.ascontiguousarray(
                lhsp.reshape(10, npair * N_ATOM)),
            "rhsr_pack": np.ascontiguousarray(
                rhsp.reshape(10, npair * 2 * N_ATOM)),
            "rgs_pack": rgsp,
            "bt_pack": btpk,
            "bt1_pack": to_fp16(btpk * f32(RB1)),
            "bt2_pack": to_fp16(btpk * f32(RB2)),
            "shS_pack": shSp,
            "q_pack": qp,
        })
    return in_maps


def run_device(in_maps, trace=False, **kw):
    if "nc" not in _CACHE:
        _CACHE["nc"] = _build_bass()
    nc = _CACHE["nc"]
    return run_bass_kernel_spmd(nc, in_maps, list(range(N_CORES)),
                                trace=trace, **kw)


def kernel(eneg, positions, node_attrs, hardness, total_charge, batch,
           atomic_numbers):
    in_maps = _host_pack(eneg, positions, node_attrs, hardness, total_charge,
                         atomic_numbers)
    res = run_device(in_maps)
    outs = []
    for c in range(N_CORES):
        o = res.results[c]["out"]                # [atom, mol]
        outs.append(np.ascontiguousarray(o.T))   # [mol, atom]
    full = np.concatenate(outs, axis=0).reshape(-1).astype(np.float32)
    return full


# revision 15
# speedup vs baseline: 1.1485x; 1.1485x over previous
"""Charge-equilibration kernel for Trainium2 (8 NeuronCores, SPMD) — v2.

Problem: 1024 molecules x 128 atoms. Per molecule build the erf-screened
Coulomb matrix A, solve the augmented system via Schur complement, return
partial charges [131072] f32.

v2 algorithm (per core: 128 molecules, data-parallel across cores):
  - Jacobi-scaled system At = D_s A D_s (unit diagonal), M = At/S0.
  - Host packs scaled build tensors so the device computes
      d~^2 = d^2 * S0^2/(s_i s_j)^2  (PE, paired-molecule f32r matmuls)
      dt  = sqrt(d~^2)               (ACT)
      rds = 1/dt = s_i s_j/(S0 d)    (DVE reciprocal)
      x   = dt * rgs                 (Pool; rgs = s_i s_j/(S0*sqrt(2)gam),
                                      f32 host pack, diag 0 -> Gf diag 0)
      ev  = erf(x)                   (ACT)
      Gf  = ev * rds                 (Pool; f32 == offdiag(At)/S0)
      g   = bf16(Gf)
  - Cubic Chebyshev seed on [a,b]:
      E0 = a0 I + a1 G + a2 G^2 + a3 G^3,  X0 = b0 I + b1 G + b2 G^2
    built via PE PSUM accumulation (G2k = bf16(a3*G^2) via ACT scale-copy).
  - K-1 product-form NS squarings E_{j+1} = E_j^2 (bf16), chain applies
    to thin rhs ([128, 2/molecule]) + R refinements against exact f32 Gf.
  - Schur: lam = (Q + sum v)/(1 - sum u), q = -(v + lam*u).

Emission is software-pipelined over cohorts of 4 superquads (8 molecules
each, [128,1024] tiles) with phase-batched ACT table usage.
"""

import os
import numpy as np

import concourse.bass as bass
import concourse.bacc as bacc
import concourse.tile as tile
import concourse.mybir as mybir
from concourse.bass_utils import run_bass_kernel_spmd
from concourse.masks import make_identity

dt = mybir.dt
AF = mybir.ActivationFunctionType
ALU = mybir.AluOpType

N_CORES = 8
B_MOL = 1024
N_ATOM = 128
MPC = B_MOL // N_CORES          # molecules per core = 128
SQ = 8                          # molecules per superquad
NSQ = MPC // SQ                 # 16 superquads
CSQ = 4                         # superquads per cohort
NCOH = NSQ // CSQ               # 4 cohorts
NM = CSQ * SQ                   # molecules per cohort = 32
W = SQ * N_ATOM                 # superquad tile width = 1024

S0 = float(os.environ.get("KE_S0", "32.5"))
CH_A = float(os.environ.get("KE_A", "0.0425"))
CH_B = float(os.environ.get("KE_B", "34.5"))
K_NS = int(os.environ.get("KE_K", "6"))     # chain length (K-1 squarings)
N_REF = int(os.environ.get("KE_R", "2"))
OFF = int(os.environ.get("KE_OFF", "12"))   # cohort pipeline offset (ticks)
EPS_D2 = 1.0e-4
L_DIAG = 1.0e10
SQRT_L = float(np.sqrt(L_DIAG))
C0 = 1.0 / S0

# engine knobs: a=ACT, v=DVE, p=Pool
E_SE = os.environ.get("KE_SE", "a")             # E0 evacuation
ECOPY = os.environ.get("KE_ECOPY", "avava")     # squaring evacs (K_NS-1)
E_GBF = os.environ.get("KE_GBF", "v")           # g = bf16(Gf)
E_CV0 = os.environ.get("KE_CV0", "v")           # cv0 thin evac
SPLIT_MUL = os.environ.get("KE_SPLIT", "1") == "1"
BUILD_F32R = os.environ.get("KE_BF32R", "0") == "1"


def _cheb_seed(a, b, deg, S0v):
    import numpy.polynomial.chebyshev as C
    import numpy.polynomial.polynomial as P
    from math import comb
    cheb = np.zeros(deg + 1)
    cheb[deg] = 1.0
    tpoly = C.cheb2poly(cheb)
    u0 = (b + a) / (b - a)
    u1 = -2.0 / (b - a)
    t_lam = np.zeros(deg + 1)
    for k in range(deg + 1):
        if tpoly[k] == 0.0:
            continue
        binom = P.polypow([u0, u1], k) if k > 0 else np.array([1.0])
        t_lam[:len(binom)] += tpoly[k] * binom
    t0 = np.polynomial.chebyshev.chebval(u0, cheb)
    e_lam = t_lam / t0
    x_lam = -e_lam[1:]
    alpha = np.zeros(deg + 1)
    beta = np.zeros(deg)
    c0 = 1.0 / S0v
    for k in range(deg + 1):
        ck = e_lam[k] * S0v ** k
        for j in range(k + 1):
            alpha[j] += ck * comb(k, j) * c0 ** (k - j)
    for k in range(deg):
        ck = x_lam[k] * S0v ** k
        for j in range(k + 1):
            beta[j] += ck * comb(k, j) * c0 ** (k - j)
    beta *= S0v
    return alpha, beta


ALPHA, BETA = _cheb_seed(CH_A, CH_B, 3, S0)
SCL3 = float(ALPHA[3])          # exact f32 scale used in G2k ACT copy
RB1 = float(BETA[1])            # rhs1 scale
RB2 = float(BETA[2] / SCL3)     # rhs2 scale (G2k-term)


def _bf_split(v):
    hi = float(np.float32(v).astype(np.float16).astype(np.float32))
    lo = float(np.float32(v - hi))
    return hi, lo


A1_HI, A1_LO = _bf_split(ALPHA[1])
R23_HI, R23_LO = _bf_split(ALPHA[2] / SCL3)

_CACHE = {}


def _build_bass():
    nc = bacc.Bacc()
    f32 = dt.float32
    bf = dt.float16
    bfL = dt.bfloat16

    NPAIR = MPC // 2
    lhs = nc.declare_dram_parameter("lhsr_pack", [10, NPAIR * N_ATOM], f32,
                                    isOutput=False)
    rhs = nc.declare_dram_parameter("rhsr_pack", [10, NPAIR * 2 * N_ATOM],
                                    f32, isOutput=False)
    rgs = nc.declare_dram_parameter("rgs_pack", [N_ATOM, MPC * N_ATOM], f32,
                                    isOutput=False)
    btp = nc.declare_dram_parameter("bt_pack", [N_ATOM, 2 * MPC], f32,
                                    isOutput=False)
    bt1p = nc.declare_dram_parameter("bt1_pack", [N_ATOM, 2 * MPC], bf,
                                     isOutput=False)
    bt2p = nc.declare_dram_parameter("bt2_pack", [N_ATOM, 2 * MPC], bf,
                                     isOutput=False)
    shS = nc.declare_dram_parameter("shS_pack", [N_ATOM, 2 * MPC], f32,
                                    isOutput=False)
    qpk = nc.declare_dram_parameter("q_pack", [1, MPC], f32, isOutput=False)
    out = nc.declare_dram_parameter("out", [N_ATOM, MPC], f32, isOutput=True)
    DBG = os.environ.get("KE_DBG", "") == "1"
    dbg = {}
    if DBG:
        for nm, w_ in [("d_dt", W), ("d_rds", W), ("d_ev", W), ("d_gf", W),
                       ("d_g2k", W), ("d_se", W), ("d_e1", W), ("d_w0", 64),
                       ("d_w1", 64)]:
            dbg[nm] = nc.declare_dram_parameter(nm, [N_ATOM, w_], f32,
                                                isOutput=True)

    def cp(eng, dst, src):
        if eng == "a":
            nc.scalar.copy(dst, src)
        elif eng == "v":
            nc.vector.tensor_copy(dst, src)
        else:
            nc.gpsimd.tensor_copy(dst, src)

    from contextlib import ExitStack

    with tile.TileContext(nc) as tc:
        with ExitStack() as es:
            def pool(name, bufs, space=None):
                kw = {"space": space} if space else {}
                return es.enter_context(
                    tc.tile_pool(name=name, bufs=bufs, **kw))

            const = pool("const", 1)
            core_in = pool("core_in", 1)
            DP = int(os.environ.get("KE_DP", "3"))
            lhs_in = pool("lhs_in", DP)
            rhs_in = pool("rhs_in", DP)
            rgs_in = pool("rgs_in", DP + 1)
            dtp = pool("dtp", DP)
            rdsp = pool("rdsp", DP)
            xp = pool("xp", DP)
            gfp = pool("gfp", CSQ + 2)
            gp = pool("gp", CSQ + 2)
            g2kp = pool("g2kp", CSQ + 2)
            sep = pool("sep", K_NS * CSQ + 4)
            cvp = pool("cvp", 4)
            wp = pool("wp", 5)
            fint = pool("fint", 8)
            lamp = pool("lamp", 8)
            p_wide = pool("p_wide", 3, "PSUM")
            p_thin = pool("p_thin", 2, "PSUM")

            # ---- constants ----
            identI = const.tile([128, 128], bf)
            make_identity(nc, identI[:])          # plain identity (for k1blk)
            k2I = const.tile([128, 128], bf)
            nc.gpsimd.memset(k2I[:], A1_HI)
            nc.gpsimd.affine_select(
                out=k2I[:], in_=k2I[:], compare_op=ALU.is_equal,
                fill=0.0, base=0, pattern=[[-1, 128]], channel_multiplier=1)
            r23I = const.tile([128, 128], bf)
            nc.gpsimd.memset(r23I[:], R23_HI)
            nc.gpsimd.affine_select(
                out=r23I[:], in_=r23I[:], compare_op=ALU.is_equal,
                fill=0.0, base=0, pattern=[[-1, 128]], channel_multiplier=1)
            k2lo = const.tile([128, 128], bf)
            nc.gpsimd.memset(k2lo[:], A1_LO)
            nc.gpsimd.affine_select(
                out=k2lo[:], in_=k2lo[:], compare_op=ALU.is_equal,
                fill=0.0, base=0, pattern=[[-1, 128]], channel_multiplier=1)
            r23lo = const.tile([128, 128], bf)
            nc.gpsimd.memset(r23lo[:], R23_LO)
            nc.gpsimd.affine_select(
                out=r23lo[:], in_=r23lo[:], compare_op=ALU.is_equal,
                fill=0.0, base=0, pattern=[[-1, 128]], channel_multiplier=1)
            b0I = const.tile([128, 128], bf)
            nc.gpsimd.memset(b0I[:], float(BETA[0]))
            nc.gpsimd.affine_select(
                out=b0I[:], in_=b0I[:], compare_op=ALU.is_equal,
                fill=0.0, base=0, pattern=[[-1, 128]], channel_multiplier=1)
            identL = const.tile([128, 128], bfL)
            nc.gpsimd.memset(identL[:], SQRT_L)
            nc.gpsimd.affine_select(
                out=identL[:], in_=identL[:], compare_op=ALU.is_equal,
                fill=0.0, base=0, pattern=[[-1, 128]], channel_multiplier=1)
            k1blk = const.tile([128, W], bf)
            nc.gpsimd.memset(k1blk[:], float(ALPHA[0]))
            nc.gpsimd.affine_select(
                out=k1blk[:], in_=k1blk[:], compare_op=ALU.is_equal,
                fill=0.0, base=0, pattern=[[0, SQ], [-1, 128]],
                channel_multiplier=1)
            ones_col = const.tile([128, 1], f32)
            nc.gpsimd.memset(ones_col[:], 1.0)

            # ---- whole-core small inputs ----
            bt_all = core_in.tile([N_ATOM, 2 * MPC], f32)
            nc.sync.dma_start(bt_all[:], btp[:])
            bt1_all = core_in.tile([N_ATOM, 2 * MPC], bf)
            nc.sync.dma_start(bt1_all[:], bt1p[:])
            bt2_all = core_in.tile([N_ATOM, 2 * MPC], bf)
            nc.sync.dma_start(bt2_all[:], bt2p[:])
            shS_all = core_in.tile([N_ATOM, 2 * MPC], f32)
            nc.sync.dma_start(shS_all[:], shS[:])
            q_all = core_in.tile([1, MPC], f32)
            nc.sync.dma_start(q_all[:], qpk[:])
            btb_all = core_in.tile([N_ATOM, 2 * MPC], bf)
            nc.vector.tensor_copy(btb_all[:], bt_all[:])

            # ---- phases (cohort state dicts) ----
            def ph_build(st, c):
                st["pd"] = {}
                st["dt"] = {}
                for si in range(CSQ):
                    sq = c * CSQ + si
                    lh = lhs_in.tile([10, 4 * 128], f32, tag="lh")
                    nc.sync.dma_start(
                        lh[:], lhs[:, sq * 4 * 128:(sq + 1) * 4 * 128])
                    rh = rhs_in.tile([10, 4 * 256], f32, tag="rh")
                    nc.sync.dma_start(
                        rh[:], rhs[:, sq * 4 * 256:(sq + 1) * 4 * 256])
                    rg = rgs_in.tile([128, W], f32, tag="rg")
                    nc.sync.dma_start(
                        rg[:], rgs[:, sq * W:(sq + 1) * W])
                    st.setdefault("rg", {})[si] = rg
                    pd = p_wide.tile([128, W], f32, tag="w")
                    for p in range(4):      # pairs of molecules
                        lsl = lh[:, p * 128:(p + 1) * 128]
                        rsl = rh[:, p * 256:(p + 1) * 256]
                        if BUILD_F32R:
                            lsl = lsl.bitcast(dt.float32r)
                            rsl = rsl.bitcast(dt.float32r)
                        ps = slice(p * 256, (p + 1) * 256)
                        nc.tensor.matmul(pd[:, ps], lsl, rsl,
                                         start=True, stop=False)
                        nc.tensor.matmul(
                            pd[:, p * 256:p * 256 + 128], identL[:],
                            identL[:], start=False, stop=False)
                        nc.tensor.matmul(
                            pd[:, p * 256 + 128:(p + 1) * 256], identL[:],
                            identL[:], start=False, stop=True)
                    st["pd"][si] = pd

            def ph_sqrt(st, c):
                for si in range(CSQ):
                    pd = st["pd"].pop(si)
                    dts = dtp.tile([128, W], f32, tag="dt")
                    nc.scalar.sqrt(dts[:], pd[:])
                    st["dt"][si] = dts
                    if DBG and c == 0 and si == 3:
                        nc.sync.dma_start(dbg["d_dt"][:], dts[:])

            def ph_rx(st, c):
                st["rds"] = {}
                st["x"] = {}
                for si in range(CSQ):
                    dts = st["dt"][si]
                    rds = rdsp.tile([128, W], f32, tag="rds")
                    nc.vector.reciprocal(rds[:], dts[:])
                    st["rds"][si] = rds
                    x = xp.tile([128, W], f32, tag="x")
                    rg = st["rg"][si]
                    if SPLIT_MUL:
                        h = W // 2
                        nc.gpsimd.tensor_mul(x[:, 0:h], dts[:, 0:h],
                                             rg[:, 0:h])
                        nc.vector.tensor_mul(x[:, h:W], dts[:, h:W],
                                             rg[:, h:W])
                    else:
                        nc.gpsimd.tensor_mul(x[:], dts[:], rg[:])
                    st["x"][si] = x
                st["dt"].clear()

            def ph_erf(st, c):
                for si in range(CSQ):
                    x = st["x"][si]
                    nc.scalar.activation(x[:], x[:], AF.Erf)   # in-place

            def ph_gf(st, c):
                st["gf"] = {}
                st["g"] = {}
                for si in range(CSQ):
                    x = st["x"].pop(si)
                    rds = st["rds"].pop(si)
                    gf = gfp.tile([128, W], f32, tag="gf")
                    if SPLIT_MUL:
                        h = W // 2
                        nc.gpsimd.tensor_mul(gf[:, 0:h], x[:, 0:h],
                                             rds[:, 0:h])
                        nc.vector.tensor_mul(gf[:, h:W], x[:, h:W],
                                             rds[:, h:W])
                    else:
                        nc.gpsimd.tensor_mul(gf[:], x[:], rds[:])
                    st["gf"][si] = gf
                    if DBG and c == 0 and si == 3:
                        nc.sync.dma_start(dbg["d_rds"][:], rds[:])
                        nc.sync.dma_start(dbg["d_ev"][:], x[:])
                        nc.sync.dma_start(dbg["d_gf"][:], gf[:])
                    g = gp.tile([128, W], bf, tag="g")
                    cp(E_GBF, g[:], gf[:])
                    st["g"][si] = g
                st["rg"].clear()

            def ph_warm(st, c):
                # sub-loops per stage so PE/ACT pipeline across superquads
                st["es"] = {}
                st["g2k"] = {}
                pbs, pb2s = {}, {}
                for si in range(CSQ):
                    g = st["g"][si]
                    pb = p_wide.tile([128, W], f32, tag="w")
                    for m in range(SQ):
                        sl = slice(m * 128, (m + 1) * 128)
                        nc.tensor.matmul(pb[:, sl], g[:, sl], g[:, sl],
                                         start=True, stop=True)
                    pbs[si] = pb
                    g2k = g2kp.tile([128, W], bf, tag="g2k")
                    nc.scalar.activation(g2k[:], pb[:], AF.Copy, scale=SCL3)
                    st["g2k"][si] = g2k
                for si in range(CSQ):
                    g = st["g"][si]
                    g2k = st["g2k"][si]
                    pb2 = p_wide.tile([128, W], f32, tag="w")
                    for h in range(2):     # two 512-col halves (PSUM banks)
                        hs = slice(h * 512, (h + 1) * 512)
                        nc.tensor.matmul(pb2[:, hs], k2I[:], g[:, hs],
                                         start=True, stop=False)
                        nc.tensor.matmul(pb2[:, hs], k2lo[:], g[:, hs],
                                         start=False, stop=False)
                        nc.tensor.matmul(pb2[:, hs], r23I[:], g2k[:, hs],
                                         start=False, stop=False)
                        nc.tensor.matmul(pb2[:, hs], r23lo[:], g2k[:, hs],
                                         start=False, stop=False)
                        for m in range(4 * h, 4 * h + 4):
                            sl = slice(m * 128, (m + 1) * 128)
                            nc.tensor.matmul(pb2[:, sl], g2k[:, sl],
                                             g[:, sl], start=False,
                                             stop=False)
                        nc.tensor.matmul(pb2[:, hs], identI[:],
                                         k1blk[:, hs], start=False,
                                         stop=True)
                    pb2s[si] = pb2
                    se = sep.tile([128, W], bf, tag="se")
                    cp(E_SE, se[:], pb2[:])
                    st["es"][si] = [se]
                    if DBG and c == 0 and si == 3:
                        tmp = xp.tile([128, W], f32, tag="x")
                        nc.vector.tensor_copy(tmp[:], st["g2k"][si][:])
                        nc.sync.dma_start(dbg["d_g2k"][:], tmp[:])
                        tmp2 = xp.tile([128, W], f32, tag="x")
                        nc.vector.tensor_copy(tmp2[:], se[:])
                        nc.sync.dma_start(dbg["d_se"][:], tmp2[:])

            def ph_ns(st, c, k):
                for si in range(CSQ):
                    se = st["es"][si][-1]
                    eb = p_wide.tile([128, W], f32, tag="w")
                    for m in range(SQ):
                        sl = slice(m * 128, (m + 1) * 128)
                        nc.tensor.matmul(eb[:, sl], se[:, sl], se[:, sl],
                                         start=True, stop=True)
                    se2 = sep.tile([128, W], bf, tag="se")
                    cp(ECOPY[k], se2[:], eb[:])
                    st["es"][si].append(se2)
                    if DBG and c == 0 and si == 3 and k == 0:
                        tmp = xp.tile([128, W], f32, tag="x")
                        nc.vector.tensor_copy(tmp[:], se2[:])
                        nc.sync.dma_start(dbg["d_e1"][:], tmp[:])

            def emit_apply(st, c, rhs_bf, rhs1, rhs2, w_prev):
                """w = (w_prev +) chain(rhs): X0 = b0 I + b1 G + (b2/a3) G2k,
                then K stages of (I+E_j). rhs1 = RB1*rhs, rhs2 = RB2*rhs."""
                gb = p_thin.tile([128, 2 * NM], f32, tag="t")
                for mi in range(NM):
                    si, m = mi // SQ, mi % SQ
                    sl = slice(m * 128, (m + 1) * 128)
                    ts = slice(2 * mi, 2 * mi + 2)
                    nc.tensor.matmul(gb[:, ts], b0I[:], rhs_bf[:, ts],
                                     start=True, stop=False)
                    nc.tensor.matmul(gb[:, ts], st["g"][si][:, sl],
                                     rhs1[:, ts], start=False, stop=False)
                    nc.tensor.matmul(gb[:, ts], st["g2k"][si][:, sl],
                                     rhs2[:, ts], start=False, stop=True)
                cv = cvp.tile([128, 2 * NM], bf, tag="cv")
                cp(E_CV0, cv[:], gb[:])
                for j in range(K_NS):
                    cb = p_thin.tile([128, 2 * NM], f32, tag="t")
                    for mi in range(NM):
                        si, m = mi // SQ, mi % SQ
                        sl = slice(m * 128, (m + 1) * 128)
                        nc.tensor.matmul(
                            cb[:, 2 * mi:2 * mi + 2],
                            st["es"][si][j][:, sl],
                            cv[:, 2 * mi:2 * mi + 2],
                            start=(mi == 0), stop=(mi == NM - 1))
                    if j < K_NS - 1:
                        cv2 = cvp.tile([128, 2 * NM], bf, tag="cv")
                        nc.vector.tensor_add(cv2[:], cv[:], cb[:])
                        cv = cv2
                    else:
                        w = wp.tile([128, 2 * NM], f32, tag="w")
                        if w_prev is None:
                            nc.vector.tensor_add(w[:], cv[:], cb[:])
                        else:
                            cv3 = fint.tile([128, 2 * NM], f32, tag="cv3")
                            nc.vector.tensor_add(cv3[:], cv[:], cb[:])
                            nc.vector.tensor_add(w[:], w_prev[:], cv3[:])
                return w

            def ph_fa(st, c):
                csl = slice(c * 2 * NM, (c + 1) * 2 * NM)
                st["w"] = emit_apply(st, c, btb_all[:, csl],
                                     bt1_all[:, csl], bt2_all[:, csl], None)
                if DBG and c == 0:
                    nc.sync.dma_start(dbg["d_w0"][:], st["w"][:])

            def ph_fr(st, c):
                csl = slice(c * 2 * NM, (c + 1) * 2 * NM)
                w = st["w"]
                t2 = fint.tile([128, 2 * NM], f32, tag="t2")
                nc.vector.scalar_tensor_tensor(
                    out=t2[:], in0=w[:], scalar=-C0, in1=bt_all[:, csl],
                    op0=ALU.mult, op1=ALU.add)
                pp = p_thin.tile([128, 2 * NM], f32, tag="t")
                for mi in range(NM):
                    si, m = mi // SQ, mi % SQ
                    sl = slice(m * 128, (m + 1) * 128)
                    nc.tensor.matmul(pp[:, 2 * mi:2 * mi + 2],
                                     st["gf"][si][:, sl],
                                     w[:, 2 * mi:2 * mi + 2],
                                     start=(mi == 0), stop=(mi == NM - 1))
                rt = fint.tile([128, 2 * NM], bf, tag="rt")
                nc.vector.scalar_tensor_tensor(
                    out=rt[:], in0=pp[:], scalar=-1.0, in1=t2[:],
                    op0=ALU.mult, op1=ALU.add)
                rt1 = fint.tile([128, 2 * NM], bf, tag="rt1")
                nc.vector.tensor_scalar_mul(rt1[:], rt[:], RB1)
                rt2 = fint.tile([128, 2 * NM], bf, tag="rt2")
                nc.vector.tensor_scalar_mul(rt2[:], rt[:], RB2)
                st["w"] = emit_apply(st, c, rt, rt1, rt2, w)
                if DBG and c == 0:
                    nc.sync.dma_start(dbg["d_w1"][:], st["w"][:])

            def ph_fs(st, c):
                csl = slice(c * 2 * NM, (c + 1) * 2 * NM)
                ws = lamp.tile([128, 2 * NM], f32, tag="ws")
                nc.vector.tensor_mul(ws[:], st["w"][:], shS_all[:, csl])
                sums = p_thin.tile([1, 2 * NM], f32, tag="t")
                nc.tensor.matmul(sums[:], ones_col[:], ws[:])
                num = lamp.tile([1, NM], f32, tag="num")
                nc.vector.tensor_add(
                    num[:], sums[0:1, 0:2 * NM:2],
                    q_all[:, c * NM:(c + 1) * NM])
                den = lamp.tile([1, NM], f32, tag="den")
                nc.vector.tensor_scalar_add(den[:], sums[0:1, 1:2 * NM:2],
                                            -1.0)
                rden = lamp.tile([1, NM], f32, tag="rden")
                nc.vector.reciprocal(rden[:], den[:])
                lamneg = lamp.tile([1, NM], f32, tag="lamneg")
                nc.vector.tensor_mul(lamneg[:], num[:], rden[:])
                lamb = lamp.tile([128, NM], f32, tag="lamb")
                nc.gpsimd.partition_broadcast(lamb[:], lamneg[:])
                t1 = lamp.tile([128, NM], f32, tag="t1")
                nc.vector.tensor_mul(t1[:], ws[:, 1:2 * NM:2], lamb[:])
                qc = lamp.tile([128, NM], f32, tag="qc")
                nc.vector.tensor_sub(qc[:], t1[:], ws[:, 0:2 * NM:2])
                nc.sync.dma_start(out[:, c * NM:(c + 1) * NM], qc[:])
                st["es"].clear()
                st["gf"].clear()
                st["g"].clear()
                st["g2k"].clear()

            # phase table
            def emit_phase(st, c, ph):
                if ph == 0:
                    ph_build(st, c)
                elif ph == 1:
                    ph_sqrt(st, c)
                elif ph == 2:
                    ph_rx(st, c)
                elif ph == 3:
                    ph_erf(st, c)
                elif ph == 4:
                    ph_gf(st, c)
                elif ph == 5:
                    ph_warm(st, c)
                elif ph < 5 + K_NS:
                    ph_ns(st, c, ph - 6)
                elif ph == 5 + K_NS:
                    ph_fa(st, c)
                elif ph < 6 + K_NS + N_REF:
                    ph_fr(st, c)
                else:
                    ph_fs(st, c)

            NPH = 7 + K_NS + N_REF
            states = [dict() for _ in range(NCOH)]
            total = OFF * (NCOH - 1) + NPH
            for t in range(total):
                for c in range(NCOH):
                    ph = t - OFF * c
                    if 0 <= ph < NPH:
                        emit_phase(states[c], c, ph)

    nc.compile()
    return nc


def _host_pack(eneg, positions, node_attrs, hardness, total_charge,
               atomic_numbers):
    """Precompute per-atom quantities and pack per-core DRAM tensors."""
    f32 = np.float32
    pos = np.ascontiguousarray(positions, dtype=f32).reshape(B_MOL, N_ATOM, 3)
    Z = np.asarray(atomic_numbers).astype(np.int64).reshape(B_MOL, N_ATOM)
    na = np.asarray(node_attrs, dtype=f32).reshape(B_MOL, N_ATOM, -1)
    hard = np.asarray(hardness, dtype=f32)
    e = np.asarray(eneg, dtype=f32).reshape(B_MOL, N_ATOM)
    Q = np.asarray(total_charge, dtype=f32).reshape(B_MOL)

    cov = (0.3 + 0.02 * np.arange(100)).astype(f32)
    r = cov[Z]                                   # [B, n]
    sig = (r * r).astype(f32)
    n2 = (pos * pos).sum(axis=2, dtype=f32).astype(f32)
    aidx = na.argmax(axis=2)
    dv = (hard[aidx] + f32(1.0) / (np.sqrt(np.pi).astype(f32) * r)).astype(f32)
    sh = (f32(1.0) / np.sqrt(dv)).astype(f32)    # s = 1/sqrt(diag A)

    def to_fp16(x):
        return np.ascontiguousarray(
            np.asarray(x, dtype=np.float32).astype(np.float16))

    mpc = MPC
    npair = mpc // 2
    in_maps = []
    for c in range(N_CORES):
        sl = slice(c * mpc, (c + 1) * mpc)
        p = pos[sl]          # [mpc, 128, 3]
        nn2 = n2[sl]
        sgl = sig[sl]
        shl = sh[sl]         # [mpc, 128]
        el = e[sl]

        F = (f32(S0) / (shl * shl)).astype(f32)       # S0/s^2  [mpc, n]
        # per-molecule scaled lhs rows [5, n] and rhs rows [5, n]
        lhs5 = np.stack([-2.0 * p[:, :, 0] * F, -2.0 * p[:, :, 1] * F,
                         -2.0 * p[:, :, 2] * F, (nn2 + EPS_D2) * F, F],
                        axis=1).astype(f32)            # [mpc, 5, n]
        rhs5 = np.stack([p[:, :, 0] * F, p[:, :, 1] * F, p[:, :, 2] * F,
                         F, nn2 * F], axis=1).astype(f32)

        lhsp = np.zeros((10, npair, N_ATOM), dtype=f32)
        lhsp[0:5] = lhs5[0::2].transpose(1, 0, 2)
        lhsp[5:10] = lhs5[1::2].transpose(1, 0, 2)
        rhsp = np.zeros((10, npair, 2 * N_ATOM), dtype=f32)
        rhsp[0:5, :, :N_ATOM] = rhs5[0::2].transpose(1, 0, 2)
        rhsp[5:10, :, N_ATOM:] = rhs5[1::2].transpose(1, 0, 2)

        # rgs = s_i s_j / (S0 * sqrt(2 sig_i + 2 sig_j)), diag 0
        gam2 = 2.0 * (sgl[:, :, None] + sgl[:, None, :])
        rgsp = (np.einsum("mi,mj->mij", shl, shl)
                / (f32(S0) * np.sqrt(gam2))).astype(f32)
        ii = np.arange(N_ATOM)
        rgsp[:, ii, ii] = 0.0
        rgsp = np.ascontiguousarray(
            rgsp.transpose(1, 0, 2).reshape(N_ATOM, mpc * N_ATOM))

        btpk = np.empty((N_ATOM, 2 * mpc), dtype=f32)
        btpk[:, 0::2] = (el * shl / f32(S0)).T
        btpk[:, 1::2] = (shl / f32(S0)).T
        shSp = np.empty((N_ATOM, 2 * mpc), dtype=f32)
        shSp[:, 0::2] = shl.T
        shSp[:, 1::2] = shl.T
        qp = np.ascontiguousarray(Q[sl]).reshape(1, mpc)
        in_maps.append({
            "lhsr_pack": np.ascontiguousarray(
                lhsp.reshape(10, npair * N_ATOM)),
            "rhsr_pack": np.ascontiguousarray(
                rhsp.reshape(10, npair * 2 * N_ATOM)),
            "rgs_pack": rgsp,
            "bt_pack": btpk,
            "bt1_pack": to_fp16(btpk * f32(RB1)),
            "bt2_pack": to_fp16(btpk * f32(RB2)),
            "shS_pack": shSp,
            "q_pack": qp,
        })
    return in_maps


def run_device(in_maps, trace=False, **kw):
    if "nc" not in _CACHE:
        _CACHE["nc"] = _build_bass()
    nc = _CACHE["nc"]
    return run_bass_kernel_spmd(nc, in_maps, list(range(N_CORES)),
                                trace=trace, **kw)


def kernel(eneg, positions, node_attrs, hardness, total_charge, batch,
           atomic_numbers):
    in_maps = _host_pack(eneg, positions, node_attrs, hardness, total_charge,
                         atomic_numbers)
    res = run_device(in_maps)
    outs = []
    for c in range(N_CORES):
        o = res.results[c]["out"]                # [atom, mol]
        outs.append(np.ascontiguousarray(o.T))   # [mol, atom]
    full = np.concatenate(outs, axis=0).reshape(-1).astype(np.float32)
    return full


# revision 18
# speedup vs baseline: 1.1880x; 1.0344x over previous
"""Charge-equilibration kernel for Trainium2 (8 NeuronCores, SPMD) — v2.

Problem: 1024 molecules x 128 atoms. Per molecule build the erf-screened
Coulomb matrix A, solve the augmented system via Schur complement, return
partial charges [131072] f32.

v2 algorithm (per core: 128 molecules, data-parallel across cores):
  - Jacobi-scaled system At = D_s A D_s (unit diagonal), M = At/S0.
  - Host packs scaled build tensors so the device computes
      d~^2 = d^2 * S0^2/(s_i s_j)^2  (PE, paired-molecule f32r matmuls)
      dt  = sqrt(d~^2)               (ACT)
      rds = 1/dt = s_i s_j/(S0 d)    (DVE reciprocal)
      x   = dt * rgs                 (Pool; rgs = s_i s_j/(S0*sqrt(2)gam),
                                      f32 host pack, diag 0 -> Gf diag 0)
      ev  = erf(x)                   (ACT)
      Gf  = ev * rds                 (Pool; f32 == offdiag(At)/S0)
      g   = bf16(Gf)
  - Cubic Chebyshev seed on [a,b]:
      E0 = a0 I + a1 G + a2 G^2 + a3 G^3,  X0 = b0 I + b1 G + b2 G^2
    built via PE PSUM accumulation (G2k = bf16(a3*G^2) via ACT scale-copy).
  - K-1 product-form NS squarings E_{j+1} = E_j^2 (bf16), chain applies
    to thin rhs ([128, 2/molecule]) + R refinements against exact f32 Gf.
  - Schur: lam = (Q + sum v)/(1 - sum u), q = -(v + lam*u).

Emission is software-pipelined over cohorts of 4 superquads (8 molecules
each, [128,1024] tiles) with phase-batched ACT table usage.
"""

import os
import numpy as np

import concourse.bass as bass
import concourse.bacc as bacc
import concourse.tile as tile
import concourse.mybir as mybir
from concourse.bass_utils import run_bass_kernel_spmd
from concourse.masks import make_identity

dt = mybir.dt
AF = mybir.ActivationFunctionType
ALU = mybir.AluOpType

N_CORES = 8
B_MOL = 1024
N_ATOM = 128
MPC = B_MOL // N_CORES          # molecules per core = 128
SQ = 8                          # molecules per superquad
NSQ = MPC // SQ                 # 16 superquads
CSQ = 4                         # superquads per cohort
NCOH = NSQ // CSQ               # 4 cohorts
NM = CSQ * SQ                   # molecules per cohort = 32
W = SQ * N_ATOM                 # superquad tile width = 1024

S0 = float(os.environ.get("KE_S0", "32.5"))
CH_A = float(os.environ.get("KE_A", "0.0425"))
CH_B = float(os.environ.get("KE_B", "34.5"))
K_NS = int(os.environ.get("KE_K", "6"))     # chain length (K-1 squarings)
N_REF = int(os.environ.get("KE_R", "2"))
OFF = int(os.environ.get("KE_OFF", "12"))   # cohort pipeline offset (ticks)
EPS_D2 = 1.0e-4
L_DIAG = 1.0e10
SQRT_L = float(np.sqrt(L_DIAG))
C0 = 1.0 / S0

# engine knobs: a=ACT, v=DVE, p=Pool
E_SE = os.environ.get("KE_SE", "v")             # E0 evacuation
ECOPY = os.environ.get("KE_ECOPY", "avava")     # squaring evacs (K_NS-1)
E_GBF = os.environ.get("KE_GBF", "v")           # g = bf16(Gf)
E_CV0 = os.environ.get("KE_CV0", "v")           # cv0 thin evac
SPLIT_MUL = os.environ.get("KE_SPLIT", "1") == "1"
E_G2K = os.environ.get("KE_G2K", "a")           # G2k evac engine
POOL_FRAC = float(os.environ.get("KE_PF", "0.625"))  # pool share of split muls
BUILD_F32R = os.environ.get("KE_BF32R", "0") == "1"


def _cheb_seed(a, b, deg, S0v):
    import numpy.polynomial.chebyshev as C
    import numpy.polynomial.polynomial as P
    from math import comb
    cheb = np.zeros(deg + 1)
    cheb[deg] = 1.0
    tpoly = C.cheb2poly(cheb)
    u0 = (b + a) / (b - a)
    u1 = -2.0 / (b - a)
    t_lam = np.zeros(deg + 1)
    for k in range(deg + 1):
        if tpoly[k] == 0.0:
            continue
        binom = P.polypow([u0, u1], k) if k > 0 else np.array([1.0])
        t_lam[:len(binom)] += tpoly[k] * binom
    t0 = np.polynomial.chebyshev.chebval(u0, cheb)
    e_lam = t_lam / t0
    x_lam = -e_lam[1:]
    alpha = np.zeros(deg + 1)
    beta = np.zeros(deg)
    c0 = 1.0 / S0v
    for k in range(deg + 1):
        ck = e_lam[k] * S0v ** k
        for j in range(k + 1):
            alpha[j] += ck * comb(k, j) * c0 ** (k - j)
    for k in range(deg):
        ck = x_lam[k] * S0v ** k
        for j in range(k + 1):
            beta[j] += ck * comb(k, j) * c0 ** (k - j)
    beta *= S0v
    return alpha, beta


ALPHA, BETA = _cheb_seed(CH_A, CH_B, 3, S0)
SCL3 = float(ALPHA[3])          # exact f32 scale used in G2k ACT copy
RB1 = float(BETA[1])            # rhs1 scale
RB2 = float(BETA[2] / SCL3)     # rhs2 scale (G2k-term)


def _bf_split(v):
    hi = float(np.float32(v).astype(np.float16).astype(np.float32))
    lo = float(np.float32(v - hi))
    return hi, lo


A1_HI, A1_LO = _bf_split(ALPHA[1])
R23_HI, R23_LO = _bf_split(ALPHA[2] / SCL3)

_CACHE = {}


def _build_bass():
    nc = bacc.Bacc()
    f32 = dt.float32
    bf = dt.float16
    bfL = dt.bfloat16

    NPAIR = MPC // 2
    lhs = nc.declare_dram_parameter("lhsr_pack", [10, NPAIR * N_ATOM], f32,
                                    isOutput=False)
    rhs = nc.declare_dram_parameter("rhsr_pack", [10, NPAIR * 2 * N_ATOM],
                                    f32, isOutput=False)
    rgs = nc.declare_dram_parameter("rgs_pack", [N_ATOM, MPC * N_ATOM], f32,
                                    isOutput=False)
    btp = nc.declare_dram_parameter("bt_pack", [N_ATOM, 2 * MPC], f32,
                                    isOutput=False)
    bt1p = nc.declare_dram_parameter("bt1_pack", [N_ATOM, 2 * MPC], bf,
                                     isOutput=False)
    bt2p = nc.declare_dram_parameter("bt2_pack", [N_ATOM, 2 * MPC], bf,
                                     isOutput=False)
    shS = nc.declare_dram_parameter("shS_pack", [N_ATOM, 2 * MPC], f32,
                                    isOutput=False)
    qpk = nc.declare_dram_parameter("q_pack", [1, MPC], f32, isOutput=False)
    out = nc.declare_dram_parameter("out", [N_ATOM, MPC], f32, isOutput=True)
    DBG = os.environ.get("KE_DBG", "") == "1"
    dbg = {}
    if DBG:
        for nm, w_ in [("d_dt", W), ("d_rds", W), ("d_ev", W), ("d_gf", W),
                       ("d_g2k", W), ("d_se", W), ("d_e1", W), ("d_w0", 64),
                       ("d_w1", 64)]:
            dbg[nm] = nc.declare_dram_parameter(nm, [N_ATOM, w_], f32,
                                                isOutput=True)

    def cp(eng, dst, src):
        if eng == "a":
            nc.scalar.copy(dst, src)
        elif eng == "v":
            nc.vector.tensor_copy(dst, src)
        else:
            nc.gpsimd.tensor_copy(dst, src)

    from contextlib import ExitStack

    with tile.TileContext(nc) as tc:
        with ExitStack() as es:
            def pool(name, bufs, space=None):
                kw = {"space": space} if space else {}
                return es.enter_context(
                    tc.tile_pool(name=name, bufs=bufs, **kw))

            const = pool("const", 1)
            core_in = pool("core_in", 1)
            DP = int(os.environ.get("KE_DP", "3"))
            lhs_in = pool("lhs_in", DP)
            rhs_in = pool("rhs_in", DP)
            rgs_in = pool("rgs_in", DP + 1)
            dtp = pool("dtp", DP)
            rdsp = pool("rdsp", DP)
            xp = pool("xp", DP)
            gfp = pool("gfp", CSQ + 2)
            gp = pool("gp", CSQ + 2)
            g2kp = pool("g2kp", CSQ + 2)
            sep = pool("sep", K_NS * CSQ + 4)
            cvp = pool("cvp", 4)
            wp = pool("wp", 5)
            fint = pool("fint", 8)
            lamp = pool("lamp", 8)
            p_wide = pool("p_wide", 3, "PSUM")
            p_thin = pool("p_thin", 2, "PSUM")

            # ---- constants ----
            identI = const.tile([128, 128], bf)
            make_identity(nc, identI[:])          # plain identity (for k1blk)
            k2I = const.tile([128, 128], bf)
            nc.gpsimd.memset(k2I[:], A1_HI)
            nc.gpsimd.affine_select(
                out=k2I[:], in_=k2I[:], compare_op=ALU.is_equal,
                fill=0.0, base=0, pattern=[[-1, 128]], channel_multiplier=1)
            r23I = const.tile([128, 128], bf)
            nc.gpsimd.memset(r23I[:], R23_HI)
            nc.gpsimd.affine_select(
                out=r23I[:], in_=r23I[:], compare_op=ALU.is_equal,
                fill=0.0, base=0, pattern=[[-1, 128]], channel_multiplier=1)
            k2lo = const.tile([128, 128], bf)
            nc.gpsimd.memset(k2lo[:], A1_LO)
            nc.gpsimd.affine_select(
                out=k2lo[:], in_=k2lo[:], compare_op=ALU.is_equal,
                fill=0.0, base=0, pattern=[[-1, 128]], channel_multiplier=1)
            r23lo = const.tile([128, 128], bf)
            nc.gpsimd.memset(r23lo[:], R23_LO)
            nc.gpsimd.affine_select(
                out=r23lo[:], in_=r23lo[:], compare_op=ALU.is_equal,
                fill=0.0, base=0, pattern=[[-1, 128]], channel_multiplier=1)
            b0I = const.tile([128, 128], bf)
            nc.gpsimd.memset(b0I[:], float(BETA[0]))
            nc.gpsimd.affine_select(
                out=b0I[:], in_=b0I[:], compare_op=ALU.is_equal,
                fill=0.0, base=0, pattern=[[-1, 128]], channel_multiplier=1)
            k1blk = const.tile([128, W], bf)
            nc.gpsimd.memset(k1blk[:], float(ALPHA[0]))
            nc.gpsimd.affine_select(
                out=k1blk[:], in_=k1blk[:], compare_op=ALU.is_equal,
                fill=0.0, base=0, pattern=[[0, SQ], [-1, 128]],
                channel_multiplier=1)
            ones_col = const.tile([128, 1], f32)
            nc.gpsimd.memset(ones_col[:], 1.0)

            # ---- whole-core small inputs ----
            bt_all = core_in.tile([N_ATOM, 2 * MPC], f32)
            nc.sync.dma_start(bt_all[:], btp[:])
            bt1_all = core_in.tile([N_ATOM, 2 * MPC], bf)
            nc.sync.dma_start(bt1_all[:], bt1p[:])
            bt2_all = core_in.tile([N_ATOM, 2 * MPC], bf)
            nc.sync.dma_start(bt2_all[:], bt2p[:])
            shS_all = core_in.tile([N_ATOM, 2 * MPC], f32)
            nc.sync.dma_start(shS_all[:], shS[:])
            q_all = core_in.tile([1, MPC], f32)
            nc.sync.dma_start(q_all[:], qpk[:])
            btb_all = core_in.tile([N_ATOM, 2 * MPC], bf)
            nc.vector.tensor_copy(btb_all[:], bt_all[:])

            # ---- phases (cohort state dicts) ----
            def ph_build(st, c):
                st["pd"] = {}
                st["dt"] = {}
                for si in range(CSQ):
                    sq = c * CSQ + si
                    lh = lhs_in.tile([10, 4 * 128], f32, tag="lh")
                    nc.sync.dma_start(
                        lh[:], lhs[:, sq * 4 * 128:(sq + 1) * 4 * 128])
                    rh = rhs_in.tile([10, 4 * 256], f32, tag="rh")
                    nc.sync.dma_start(
                        rh[:], rhs[:, sq * 4 * 256:(sq + 1) * 4 * 256])
                    rg = rgs_in.tile([128, W], f32, tag="rg")
                    nc.sync.dma_start(
                        rg[:], rgs[:, sq * W:(sq + 1) * W])
                    st.setdefault("rg", {})[si] = rg
                    pd = p_wide.tile([128, W], f32, tag="w")
                    for p in range(4):      # pairs of molecules
                        lsl = lh[:, p * 128:(p + 1) * 128]
                        rsl = rh[:, p * 256:(p + 1) * 256]
                        if BUILD_F32R:
                            lsl = lsl.bitcast(dt.float32r)
                            rsl = rsl.bitcast(dt.float32r)
                        ps = slice(p * 256, (p + 1) * 256)
                        nc.tensor.matmul(pd[:, ps], lsl, rsl,
                                         start=True, stop=True)
                    st["pd"][si] = pd

            def ph_sqrt(st, c):
                for si in range(CSQ):
                    pd = st["pd"].pop(si)
                    dts = dtp.tile([128, W], f32, tag="dt")
                    nc.scalar.sqrt(dts[:], pd[:])
                    st["dt"][si] = dts
                    if DBG and c == 0 and si == 3:
                        nc.sync.dma_start(dbg["d_dt"][:], dts[:])

            def ph_rx(st, c):
                st["rds"] = {}
                st["x"] = {}
                for si in range(CSQ):
                    dts = st["dt"][si]
                    rds = rdsp.tile([128, W], f32, tag="rds")
                    nc.vector.reciprocal(rds[:], dts[:])
                    st["rds"][si] = rds
                    x = xp.tile([128, W], f32, tag="x")
                    rg = st["rg"][si]
                    if SPLIT_MUL:
                        h = (int(W * POOL_FRAC) // 128) * 128
                        nc.gpsimd.tensor_mul(x[:, 0:h], dts[:, 0:h],
                                             rg[:, 0:h])
                        nc.vector.tensor_mul(x[:, h:W], dts[:, h:W],
                                             rg[:, h:W])
                    else:
                        nc.gpsimd.tensor_mul(x[:], dts[:], rg[:])
                    st["x"][si] = x
                st["dt"].clear()

            def ph_erf(st, c):
                for si in range(CSQ):
                    x = st["x"][si]
                    nc.scalar.activation(x[:], x[:], AF.Erf)   # in-place

            def ph_gf(st, c):
                st["gf"] = {}
                st["g"] = {}
                for si in range(CSQ):
                    x = st["x"].pop(si)
                    rds = st["rds"].pop(si)
                    gf = gfp.tile([128, W], f32, tag="gf")
                    if SPLIT_MUL:
                        h = (int(W * POOL_FRAC) // 128) * 128
                        nc.gpsimd.tensor_mul(gf[:, 0:h], x[:, 0:h],
                                             rds[:, 0:h])
                        nc.vector.tensor_mul(gf[:, h:W], x[:, h:W],
                                             rds[:, h:W])
                    else:
                        nc.gpsimd.tensor_mul(gf[:], x[:], rds[:])
                    st["gf"][si] = gf
                    if DBG and c == 0 and si == 3:
                        nc.sync.dma_start(dbg["d_rds"][:], rds[:])
                        nc.sync.dma_start(dbg["d_ev"][:], x[:])
                        nc.sync.dma_start(dbg["d_gf"][:], gf[:])
                    g = gp.tile([128, W], bf, tag="g")
                    cp(E_GBF, g[:], gf[:])
                    st["g"][si] = g
                st["rg"].clear()

            def ph_warm(st, c):
                # sub-loops per stage so PE/ACT pipeline across superquads
                st["es"] = {}
                st["g2k"] = {}
                pbs, pb2s = {}, {}
                for si in range(CSQ):
                    g = st["g"][si]
                    pb = p_wide.tile([128, W], f32, tag="w")
                    for m in range(SQ):
                        sl = slice(m * 128, (m + 1) * 128)
                        nc.tensor.matmul(pb[:, sl], g[:, sl], g[:, sl],
                                         start=True, stop=True)
                    pbs[si] = pb
                    g2k = g2kp.tile([128, W], bf, tag="g2k")
                    if E_G2K == "a":
                        nc.scalar.activation(g2k[:], pb[:], AF.Copy,
                                             scale=SCL3)
                    else:
                        nc.vector.tensor_scalar_mul(g2k[:], pb[:], SCL3)
                    st["g2k"][si] = g2k
                for si in range(CSQ):
                    g = st["g"][si]
                    g2k = st["g2k"][si]
                    pb2 = p_wide.tile([128, W], f32, tag="w")
                    for h in range(2):     # two 512-col halves (PSUM banks)
                        hs = slice(h * 512, (h + 1) * 512)
                        nc.tensor.matmul(pb2[:, hs], k2I[:], g[:, hs],
                                         start=True, stop=False)
                        nc.tensor.matmul(pb2[:, hs], k2lo[:], g[:, hs],
                                         start=False, stop=False)
                        nc.tensor.matmul(pb2[:, hs], r23I[:], g2k[:, hs],
                                         start=False, stop=False)
                        nc.tensor.matmul(pb2[:, hs], r23lo[:], g2k[:, hs],
                                         start=False, stop=False)
                        for m in range(4 * h, 4 * h + 4):
                            sl = slice(m * 128, (m + 1) * 128)
                            nc.tensor.matmul(pb2[:, sl], g2k[:, sl],
                                             g[:, sl], start=False,
                                             stop=False)
                        nc.tensor.matmul(pb2[:, hs], identI[:],
                                         k1blk[:, hs], start=False,
                                         stop=True)
                    pb2s[si] = pb2
                    se = sep.tile([128, W], bf, tag="se")
                    cp(E_SE, se[:], pb2[:])
                    st["es"][si] = [se]
                    if DBG and c == 0 and si == 3:
                        tmp = xp.tile([128, W], f32, tag="x")
                        nc.vector.tensor_copy(tmp[:], st["g2k"][si][:])
                        nc.sync.dma_start(dbg["d_g2k"][:], tmp[:])
                        tmp2 = xp.tile([128, W], f32, tag="x")
                        nc.vector.tensor_copy(tmp2[:], se[:])
                        nc.sync.dma_start(dbg["d_se"][:], tmp2[:])

            def ph_ns(st, c, k):
                for si in range(CSQ):
                    se = st["es"][si][-1]
                    eb = p_wide.tile([128, W], f32, tag="w")
                    for m in range(SQ):
                        sl = slice(m * 128, (m + 1) * 128)
                        nc.tensor.matmul(eb[:, sl], se[:, sl], se[:, sl],
                                         start=True, stop=True)
                    se2 = sep.tile([128, W], bf, tag="se")
                    cp(ECOPY[k], se2[:], eb[:])
                    st["es"][si].append(se2)
                    if DBG and c == 0 and si == 3 and k == 0:
                        tmp = xp.tile([128, W], f32, tag="x")
                        nc.vector.tensor_copy(tmp[:], se2[:])
                        nc.sync.dma_start(dbg["d_e1"][:], tmp[:])

            def emit_apply(st, c, rhs_bf, rhs1, rhs2, w_prev):
                """w = (w_prev +) chain(rhs): X0 = b0 I + b1 G + (b2/a3) G2k,
                then K stages of (I+E_j). rhs1 = RB1*rhs, rhs2 = RB2*rhs."""
                gb = p_thin.tile([128, 2 * NM], f32, tag="t")
                for mi in range(NM):
                    si, m = mi // SQ, mi % SQ
                    sl = slice(m * 128, (m + 1) * 128)
                    ts = slice(2 * mi, 2 * mi + 2)
                    nc.tensor.matmul(gb[:, ts], b0I[:], rhs_bf[:, ts],
                                     start=True, stop=False)
                    nc.tensor.matmul(gb[:, ts], st["g"][si][:, sl],
                                     rhs1[:, ts], start=False, stop=False)
                    nc.tensor.matmul(gb[:, ts], st["g2k"][si][:, sl],
                                     rhs2[:, ts], start=False, stop=True)
                cv = cvp.tile([128, 2 * NM], bf, tag="cv")
                cp(E_CV0, cv[:], gb[:])
                for j in range(K_NS):
                    cb = p_thin.tile([128, 2 * NM], f32, tag="t")
                    for mi in range(NM):
                        si, m = mi // SQ, mi % SQ
                        sl = slice(m * 128, (m + 1) * 128)
                        nc.tensor.matmul(
                            cb[:, 2 * mi:2 * mi + 2],
                            st["es"][si][j][:, sl],
                            cv[:, 2 * mi:2 * mi + 2],
                            start=(mi == 0), stop=(mi == NM - 1))
                    if j < K_NS - 1:
                        cv2 = cvp.tile([128, 2 * NM], bf, tag="cv")
                        nc.vector.tensor_add(cv2[:], cv[:], cb[:])
                        cv = cv2
                    else:
                        w = wp.tile([128, 2 * NM], f32, tag="w")
                        if w_prev is None:
                            nc.vector.tensor_add(w[:], cv[:], cb[:])
                        else:
                            cv3 = fint.tile([128, 2 * NM], f32, tag="cv3")
                            nc.vector.tensor_add(cv3[:], cv[:], cb[:])
                            nc.vector.tensor_add(w[:], w_prev[:], cv3[:])
                return w

            def ph_fa(st, c):
                csl = slice(c * 2 * NM, (c + 1) * 2 * NM)
                st["w"] = emit_apply(st, c, btb_all[:, csl],
                                     bt1_all[:, csl], bt2_all[:, csl], None)
                if DBG and c == 0:
                    nc.sync.dma_start(dbg["d_w0"][:], st["w"][:])

            def ph_fr(st, c):
                csl = slice(c * 2 * NM, (c + 1) * 2 * NM)
                w = st["w"]
                t2 = fint.tile([128, 2 * NM], f32, tag="t2")
                nc.vector.scalar_tensor_tensor(
                    out=t2[:], in0=w[:], scalar=-C0, in1=bt_all[:, csl],
                    op0=ALU.mult, op1=ALU.add)
                pp = p_thin.tile([128, 2 * NM], f32, tag="t")
                for mi in range(NM):
                    si, m = mi // SQ, mi % SQ
                    sl = slice(m * 128, (m + 1) * 128)
                    nc.tensor.matmul(pp[:, 2 * mi:2 * mi + 2],
                                     st["gf"][si][:, sl],
                                     w[:, 2 * mi:2 * mi + 2],
                                     start=(mi == 0), stop=(mi == NM - 1))
                rt = fint.tile([128, 2 * NM], bf, tag="rt")
                nc.vector.scalar_tensor_tensor(
                    out=rt[:], in0=pp[:], scalar=-1.0, in1=t2[:],
                    op0=ALU.mult, op1=ALU.add)
                rt1 = fint.tile([128, 2 * NM], bf, tag="rt1")
                nc.vector.tensor_scalar_mul(rt1[:], rt[:], RB1)
                rt2 = fint.tile([128, 2 * NM], bf, tag="rt2")
                nc.vector.tensor_scalar_mul(rt2[:], rt[:], RB2)
                st["w"] = emit_apply(st, c, rt, rt1, rt2, w)
                if DBG and c == 0:
                    nc.sync.dma_start(dbg["d_w1"][:], st["w"][:])

            def ph_fs(st, c):
                csl = slice(c * 2 * NM, (c + 1) * 2 * NM)
                ws = lamp.tile([128, 2 * NM], f32, tag="ws")
                nc.vector.tensor_mul(ws[:], st["w"][:], shS_all[:, csl])
                sums = p_thin.tile([1, 2 * NM], f32, tag="t")
                nc.tensor.matmul(sums[:], ones_col[:], ws[:])
                num = lamp.tile([1, NM], f32, tag="num")
                nc.vector.tensor_add(
                    num[:], sums[0:1, 0:2 * NM:2],
                    q_all[:, c * NM:(c + 1) * NM])
                den = lamp.tile([1, NM], f32, tag="den")
                nc.vector.tensor_scalar_add(den[:], sums[0:1, 1:2 * NM:2],
                                            -1.0)
                rden = lamp.tile([1, NM], f32, tag="rden")
                nc.vector.reciprocal(rden[:], den[:])
                lamneg = lamp.tile([1, NM], f32, tag="lamneg")
                nc.vector.tensor_mul(lamneg[:], num[:], rden[:])
                lamb = lamp.tile([128, NM], f32, tag="lamb")
                nc.gpsimd.partition_broadcast(lamb[:], lamneg[:])
                t1 = lamp.tile([128, NM], f32, tag="t1")
                nc.vector.tensor_mul(t1[:], ws[:, 1:2 * NM:2], lamb[:])
                qc = lamp.tile([128, NM], f32, tag="qc")
                nc.vector.tensor_sub(qc[:], t1[:], ws[:, 0:2 * NM:2])
                nc.sync.dma_start(out[:, c * NM:(c + 1) * NM], qc[:])
                st["es"].clear()
                st["gf"].clear()
                st["g"].clear()
                st["g2k"].clear()

            # phase table
            def emit_phase(st, c, ph):
                if ph == 0:
                    ph_build(st, c)
                elif ph == 1:
                    ph_sqrt(st, c)
                elif ph == 2:
                    ph_rx(st, c)
                elif ph == 3:
                    ph_erf(st, c)
                elif ph == 4:
                    ph_gf(st, c)
                elif ph == 5:
                    ph_warm(st, c)
                elif ph < 5 + K_NS:
                    ph_ns(st, c, ph - 6)
                elif ph == 5 + K_NS:
                    ph_fa(st, c)
                elif ph < 6 + K_NS + N_REF:
                    ph_fr(st, c)
                else:
                    ph_fs(st, c)

            NPH = 7 + K_NS + N_REF
            states = [dict() for _ in range(NCOH)]
            total = OFF * (NCOH - 1) + NPH
            for t in range(total):
                for c in range(NCOH):
                    ph = t - OFF * c
                    if 0 <= ph < NPH:
                        emit_phase(states[c], c, ph)

    nc.compile()
    return nc


def _host_pack(eneg, positions, node_attrs, hardness, total_charge,
               atomic_numbers):
    """Precompute per-atom quantities and pack per-core DRAM tensors."""
    f32 = np.float32
    pos = np.ascontiguousarray(positions, dtype=f32).reshape(B_MOL, N_ATOM, 3)
    Z = np.asarray(atomic_numbers).astype(np.int64).reshape(B_MOL, N_ATOM)
    na = np.asarray(node_attrs, dtype=f32).reshape(B_MOL, N_ATOM, -1)
    hard = np.asarray(hardness, dtype=f32)
    e = np.asarray(eneg, dtype=f32).reshape(B_MOL, N_ATOM)
    Q = np.asarray(total_charge, dtype=f32).reshape(B_MOL)

    cov = (0.3 + 0.02 * np.arange(100)).astype(f32)
    r = cov[Z]                                   # [B, n]
    sig = (r * r).astype(f32)
    n2 = (pos * pos).sum(axis=2, dtype=f32).astype(f32)
    aidx = na.argmax(axis=2)
    dv = (hard[aidx] + f32(1.0) / (np.sqrt(np.pi).astype(f32) * r)).astype(f32)
    sh = (f32(1.0) / np.sqrt(dv)).astype(f32)    # s = 1/sqrt(diag A)

    def to_fp16(x):
        return np.ascontiguousarray(
            np.asarray(x, dtype=np.float32).astype(np.float16))

    mpc = MPC
    npair = mpc // 2
    in_maps = []
    for c in range(N_CORES):
        sl = slice(c * mpc, (c + 1) * mpc)
        p = pos[sl]          # [mpc, 128, 3]
        nn2 = n2[sl]
        sgl = sig[sl]
        shl = sh[sl]         # [mpc, 128]
        el = e[sl]

        F = (f32(S0) / (shl * shl)).astype(f32)       # S0/s^2  [mpc, n]
        # per-molecule scaled lhs rows [5, n] and rhs rows [5, n]
        lhs5 = np.stack([-2.0 * p[:, :, 0] * F, -2.0 * p[:, :, 1] * F,
                         -2.0 * p[:, :, 2] * F, (nn2 + EPS_D2) * F, F],
                        axis=1).astype(f32)            # [mpc, 5, n]
        rhs5 = np.stack([p[:, :, 0] * F, p[:, :, 1] * F, p[:, :, 2] * F,
                         F, nn2 * F], axis=1).astype(f32)

        lhsp = np.zeros((10, npair, N_ATOM), dtype=f32)
        lhsp[0:5] = lhs5[0::2].transpose(1, 0, 2)
        lhsp[5:10] = lhs5[1::2].transpose(1, 0, 2)
        rhsp = np.zeros((10, npair, 2 * N_ATOM), dtype=f32)
        rhsp[0:5, :, :N_ATOM] = rhs5[0::2].transpose(1, 0, 2)
        rhsp[5:10, :, N_ATOM:] = rhs5[1::2].transpose(1, 0, 2)

        # rgs = s_i s_j / (S0 * sqrt(2 sig_i + 2 sig_j)), diag 0
        gam2 = 2.0 * (sgl[:, :, None] + sgl[:, None, :])
        rgsp = (np.einsum("mi,mj->mij", shl, shl)
                / (f32(S0) * np.sqrt(gam2))).astype(f32)
        ii = np.arange(N_ATOM)
        rgsp[:, ii, ii] = 0.0
        rgsp = np.ascontiguousarray(
            rgsp.transpose(1, 0, 2).reshape(N_ATOM, mpc * N_ATOM))

        btpk = np.empty((N_ATOM, 2 * mpc), dtype=f32)
        btpk[:, 0::2] = (el * shl / f32(S0)).T
        btpk[:, 1::2] = (shl / f32(S0)).T
        shSp = np.empty((N_ATOM, 2 * mpc), dtype=f32)
        shSp[:, 0::2] = shl.T
        shSp[:, 1::2] = shl.T
        qp = np.ascontiguousarray(Q[sl]).reshape(1, mpc)
        in_maps.append({
            "lhsr_pack": np.ascontiguousarray(
                lhsp.reshape(10, npair * N_ATOM)),
            "rhsr_pack": np.ascontiguousarray(
                rhsp.reshape(10, npair * 2 * N_ATOM)),
            "rgs_pack": rgsp,
            "bt_pack": btpk,
            "bt1_pack": to_fp16(btpk * f32(RB1)),
            "bt2_pack": to_fp16(btpk * f32(RB2)),
            "shS_pack": shSp,
            "q_pack": qp,
        })
    return in_maps


def run_device(in_maps, trace=False, **kw):
    if "nc" not in _CACHE:
        _CACHE["nc"] = _build_bass()
    nc = _CACHE["nc"]
    return run_bass_kernel_spmd(nc, in_maps, list(range(N_CORES)),
                                trace=trace, **kw)


def kernel(eneg, positions, node_attrs, hardness, total_charge, batch,
           atomic_numbers):
    in_maps = _host_pack(eneg, positions, node_attrs, hardness, total_charge,
                         atomic_numbers)
    res = run_device(in_maps)
    outs = []
    for c in range(N_CORES):
        o = res.results[c]["out"]                # [atom, mol]
        outs.append(np.ascontiguousarray(o.T))   # [mol, atom]
    full = np.concatenate(outs, axis=0).reshape(-1).astype(np.float32)
    return full


# revision 20
# speedup vs baseline: 1.2246x; 1.0308x over previous
"""Charge-equilibration kernel for Trainium2 (8 NeuronCores, SPMD) — v2.

Problem: 1024 molecules x 128 atoms. Per molecule build the erf-screened
Coulomb matrix A, solve the augmented system via Schur complement, return
partial charges [131072] f32.

v2 algorithm (per core: 128 molecules, data-parallel across cores):
  - Jacobi-scaled system At = D_s A D_s (unit diagonal), M = At/S0.
  - Host packs scaled build tensors so the device computes
      d~^2 = d^2 * S0^2/(s_i s_j)^2  (PE, paired-molecule f32r matmuls)
      dt  = sqrt(d~^2)               (ACT)
      rds = 1/dt = s_i s_j/(S0 d)    (DVE reciprocal)
      x   = dt * rgs                 (Pool; rgs = s_i s_j/(S0*sqrt(2)gam),
                                      f32 host pack, diag 0 -> Gf diag 0)
      ev  = erf(x)                   (ACT)
      Gf  = ev * rds                 (Pool; f32 == offdiag(At)/S0)
      g   = bf16(Gf)
  - Cubic Chebyshev seed on [a,b]:
      E0 = a0 I + a1 G + a2 G^2 + a3 G^3,  X0 = b0 I + b1 G + b2 G^2
    built via PE PSUM accumulation (G2k = bf16(a3*G^2) via ACT scale-copy).
  - K-1 product-form NS squarings E_{j+1} = E_j^2 (bf16), chain applies
    to thin rhs ([128, 2/molecule]) + R refinements against exact f32 Gf.
  - Schur: lam = (Q + sum v)/(1 - sum u), q = -(v + lam*u).

Emission is software-pipelined over cohorts of 4 superquads (8 molecules
each, [128,1024] tiles) with phase-batched ACT table usage.
"""

import os
import numpy as np

import concourse.bass as bass
import concourse.bacc as bacc
import concourse.tile as tile
import concourse.mybir as mybir
from concourse.bass_utils import run_bass_kernel_spmd
from concourse.masks import make_identity

dt = mybir.dt
AF = mybir.ActivationFunctionType
ALU = mybir.AluOpType

N_CORES = 8
B_MOL = 1024
N_ATOM = 128
MPC = B_MOL // N_CORES          # molecules per core = 128
SQ = 8                          # molecules per superquad
NSQ = MPC // SQ                 # 16 superquads
CSQ = int(os.environ.get("KE_CSQ", "4"))  # superquads per cohort
NCOH = NSQ // CSQ               # 4 cohorts
NM = CSQ * SQ                   # molecules per cohort = 32
W = SQ * N_ATOM                 # superquad tile width = 1024

S0 = float(os.environ.get("KE_S0", "32.5"))
CH_A = float(os.environ.get("KE_A", "0.0425"))
CH_B = float(os.environ.get("KE_B", "34.5"))
K_NS = int(os.environ.get("KE_K", "6"))     # chain length (K-1 squarings)
N_REF = int(os.environ.get("KE_R", "2"))
R_LIST = [int(x) for x in os.environ.get("KE_RL", "2,2,2,1").split(",")]
OFF = int(os.environ.get("KE_OFF", "12"))   # cohort pipeline offset (ticks)
EPS_D2 = 1.0e-4
L_DIAG = 1.0e10
SQRT_L = float(np.sqrt(L_DIAG))
C0 = 1.0 / S0

# engine knobs: a=ACT, v=DVE, p=Pool
E_SE = os.environ.get("KE_SE", "v")             # E0 evacuation
ECOPY = os.environ.get("KE_ECOPY", "avava")     # squaring evacs (K_NS-1)
E_GBF = os.environ.get("KE_GBF", "v")           # g = bf16(Gf)
E_CV0 = os.environ.get("KE_CV0", "v")           # cv0 thin evac
SPLIT_MUL = os.environ.get("KE_SPLIT", "1") == "1"
E_G2K = os.environ.get("KE_G2K", "a")           # G2k evac engine
POOL_FRAC = float(os.environ.get("KE_PF", "0.625"))  # pool share of split muls
BUILD_F32R = os.environ.get("KE_BF32R", "0") == "1"


def _cheb_seed(a, b, deg, S0v):
    import numpy.polynomial.chebyshev as C
    import numpy.polynomial.polynomial as P
    from math import comb
    cheb = np.zeros(deg + 1)
    cheb[deg] = 1.0
    tpoly = C.cheb2poly(cheb)
    u0 = (b + a) / (b - a)
    u1 = -2.0 / (b - a)
    t_lam = np.zeros(deg + 1)
    for k in range(deg + 1):
        if tpoly[k] == 0.0:
            continue
        binom = P.polypow([u0, u1], k) if k > 0 else np.array([1.0])
        t_lam[:len(binom)] += tpoly[k] * binom
    t0 = np.polynomial.chebyshev.chebval(u0, cheb)
    e_lam = t_lam / t0
    x_lam = -e_lam[1:]
    alpha = np.zeros(deg + 1)
    beta = np.zeros(deg)
    c0 = 1.0 / S0v
    for k in range(deg + 1):
        ck = e_lam[k] * S0v ** k
        for j in range(k + 1):
            alpha[j] += ck * comb(k, j) * c0 ** (k - j)
    for k in range(deg):
        ck = x_lam[k] * S0v ** k
        for j in range(k + 1):
            beta[j] += ck * comb(k, j) * c0 ** (k - j)
    beta *= S0v
    return alpha, beta


ALPHA, BETA = _cheb_seed(CH_A, CH_B, 3, S0)
SCL3 = float(ALPHA[3])          # exact f32 scale used in G2k ACT copy
RB1 = float(BETA[1])            # rhs1 scale
RB2 = float(BETA[2] / SCL3)     # rhs2 scale (G2k-term)


def _bf_split(v):
    hi = float(np.float32(v).astype(np.float16).astype(np.float32))
    lo = float(np.float32(v - hi))
    return hi, lo


A1_HI, A1_LO = _bf_split(ALPHA[1])
R23_HI, R23_LO = _bf_split(ALPHA[2] / SCL3)

_CACHE = {}


def _build_bass():
    nc = bacc.Bacc()
    f32 = dt.float32
    bf = dt.float16
    bfL = dt.bfloat16

    NPAIR = MPC // 2
    lhs = nc.declare_dram_parameter("lhsr_pack", [10, NPAIR * N_ATOM], f32,
                                    isOutput=False)
    rhs = nc.declare_dram_parameter("rhsr_pack", [10, NPAIR * 2 * N_ATOM],
                                    f32, isOutput=False)
    rgs = nc.declare_dram_parameter("rgs_pack", [N_ATOM, MPC * N_ATOM], f32,
                                    isOutput=False)
    btp = nc.declare_dram_parameter("bt_pack", [N_ATOM, 2 * MPC], f32,
                                    isOutput=False)
    bt1p = nc.declare_dram_parameter("bt1_pack", [N_ATOM, 2 * MPC], bf,
                                     isOutput=False)
    bt2p = nc.declare_dram_parameter("bt2_pack", [N_ATOM, 2 * MPC], bf,
                                     isOutput=False)
    shS = nc.declare_dram_parameter("shS_pack", [N_ATOM, 2 * MPC], f32,
                                    isOutput=False)
    qpk = nc.declare_dram_parameter("q_pack", [1, MPC], f32, isOutput=False)
    out = nc.declare_dram_parameter("out", [N_ATOM, MPC], f32, isOutput=True)
    DBG = os.environ.get("KE_DBG", "") == "1"
    dbg = {}
    if DBG:
        for nm, w_ in [("d_dt", W), ("d_rds", W), ("d_ev", W), ("d_gf", W),
                       ("d_g2k", W), ("d_se", W), ("d_e1", W), ("d_w0", 64),
                       ("d_w1", 64)]:
            dbg[nm] = nc.declare_dram_parameter(nm, [N_ATOM, w_], f32,
                                                isOutput=True)

    def cp(eng, dst, src):
        if eng == "a":
            nc.scalar.copy(dst, src)
        elif eng == "v":
            nc.vector.tensor_copy(dst, src)
        else:
            nc.gpsimd.tensor_copy(dst, src)

    from contextlib import ExitStack

    with tile.TileContext(nc) as tc:
        with ExitStack() as es:
            def pool(name, bufs, space=None):
                kw = {"space": space} if space else {}
                return es.enter_context(
                    tc.tile_pool(name=name, bufs=bufs, **kw))

            const = pool("const", 1)
            core_in = pool("core_in", 1)
            DP = int(os.environ.get("KE_DP", "3"))
            lhs_in = pool("lhs_in", DP)
            rhs_in = pool("rhs_in", DP)
            rgs_in = pool("rgs_in", DP + 1)
            dtp = pool("dtp", DP)
            rdsp = pool("rdsp", DP)
            xp = pool("xp", DP)
            gfp = pool("gfp", CSQ + 2)
            gp = pool("gp", CSQ + 2)
            g2kp = pool("g2kp", CSQ + 2)
            sep = pool("sep", K_NS * CSQ + 4)
            cvp = pool("cvp", 4)
            wp = pool("wp", 5)
            fint = pool("fint", 8)
            lamp = pool("lamp", 8)
            p_wide = pool("p_wide", 3, "PSUM")
            p_thin = pool("p_thin", 2, "PSUM")

            # ---- constants ----
            identI = const.tile([128, 128], bf)
            make_identity(nc, identI[:])          # plain identity (for k1blk)
            k2I = const.tile([128, 128], bf)
            nc.gpsimd.memset(k2I[:], A1_HI)
            nc.gpsimd.affine_select(
                out=k2I[:], in_=k2I[:], compare_op=ALU.is_equal,
                fill=0.0, base=0, pattern=[[-1, 128]], channel_multiplier=1)
            r23I = const.tile([128, 128], bf)
            nc.gpsimd.memset(r23I[:], R23_HI)
            nc.gpsimd.affine_select(
                out=r23I[:], in_=r23I[:], compare_op=ALU.is_equal,
                fill=0.0, base=0, pattern=[[-1, 128]], channel_multiplier=1)
            k2lo = const.tile([128, 128], bf)
            nc.gpsimd.memset(k2lo[:], A1_LO)
            nc.gpsimd.affine_select(
                out=k2lo[:], in_=k2lo[:], compare_op=ALU.is_equal,
                fill=0.0, base=0, pattern=[[-1, 128]], channel_multiplier=1)
            r23lo = const.tile([128, 128], bf)
            nc.gpsimd.memset(r23lo[:], R23_LO)
            nc.gpsimd.affine_select(
                out=r23lo[:], in_=r23lo[:], compare_op=ALU.is_equal,
                fill=0.0, base=0, pattern=[[-1, 128]], channel_multiplier=1)
            b0I = const.tile([128, 128], bf)
            nc.gpsimd.memset(b0I[:], float(BETA[0]))
            nc.gpsimd.affine_select(
                out=b0I[:], in_=b0I[:], compare_op=ALU.is_equal,
                fill=0.0, base=0, pattern=[[-1, 128]], channel_multiplier=1)
            k1blk = const.tile([128, W], bf)
            nc.gpsimd.memset(k1blk[:], float(ALPHA[0]))
            nc.gpsimd.affine_select(
                out=k1blk[:], in_=k1blk[:], compare_op=ALU.is_equal,
                fill=0.0, base=0, pattern=[[0, SQ], [-1, 128]],
                channel_multiplier=1)
            ones_col = const.tile([128, 1], f32)
            nc.gpsimd.memset(ones_col[:], 1.0)

            # ---- whole-core small inputs ----
            bt_all = core_in.tile([N_ATOM, 2 * MPC], f32)
            nc.sync.dma_start(bt_all[:], btp[:])
            bt1_all = core_in.tile([N_ATOM, 2 * MPC], bf)
            nc.sync.dma_start(bt1_all[:], bt1p[:])
            bt2_all = core_in.tile([N_ATOM, 2 * MPC], bf)
            nc.sync.dma_start(bt2_all[:], bt2p[:])
            shS_all = core_in.tile([N_ATOM, 2 * MPC], f32)
            nc.sync.dma_start(shS_all[:], shS[:])
            q_all = core_in.tile([1, MPC], f32)
            nc.sync.dma_start(q_all[:], qpk[:])
            btb_all = core_in.tile([N_ATOM, 2 * MPC], bf)
            nc.vector.tensor_copy(btb_all[:], bt_all[:])

            # ---- phases (cohort state dicts) ----
            def ph_build(st, c):
                st["pd"] = {}
                st["dt"] = {}
                for si in range(CSQ):
                    sq = c * CSQ + si
                    lh = lhs_in.tile([10, 4 * 128], f32, tag="lh")
                    nc.sync.dma_start(
                        lh[:], lhs[:, sq * 4 * 128:(sq + 1) * 4 * 128])
                    rh = rhs_in.tile([10, 4 * 256], f32, tag="rh")
                    nc.sync.dma_start(
                        rh[:], rhs[:, sq * 4 * 256:(sq + 1) * 4 * 256])
                    rg = rgs_in.tile([128, W], f32, tag="rg")
                    nc.sync.dma_start(
                        rg[:], rgs[:, sq * W:(sq + 1) * W])
                    st.setdefault("rg", {})[si] = rg
                    pd = p_wide.tile([128, W], f32, tag="w")
                    for p in range(4):      # pairs of molecules
                        lsl = lh[:, p * 128:(p + 1) * 128]
                        rsl = rh[:, p * 256:(p + 1) * 256]
                        if BUILD_F32R:
                            lsl = lsl.bitcast(dt.float32r)
                            rsl = rsl.bitcast(dt.float32r)
                        ps = slice(p * 256, (p + 1) * 256)
                        nc.tensor.matmul(pd[:, ps], lsl, rsl,
                                         start=True, stop=True)
                    st["pd"][si] = pd

            def ph_sqrt(st, c):
                for si in range(CSQ):
                    pd = st["pd"].pop(si)
                    dts = dtp.tile([128, W], f32, tag="dt")
                    nc.scalar.sqrt(dts[:], pd[:])
                    st["dt"][si] = dts
                    if DBG and c == 0 and si == 3:
                        nc.sync.dma_start(dbg["d_dt"][:], dts[:])

            def ph_rx(st, c):
                st["rds"] = {}
                st["x"] = {}
                for si in range(CSQ):
                    dts = st["dt"][si]
                    rds = rdsp.tile([128, W], f32, tag="rds")
                    nc.vector.reciprocal(rds[:], dts[:])
                    st["rds"][si] = rds
                    x = xp.tile([128, W], f32, tag="x")
                    rg = st["rg"][si]
                    if SPLIT_MUL:
                        h = (int(W * POOL_FRAC) // 128) * 128
                        nc.gpsimd.tensor_mul(x[:, 0:h], dts[:, 0:h],
                                             rg[:, 0:h])
                        nc.vector.tensor_mul(x[:, h:W], dts[:, h:W],
                                             rg[:, h:W])
                    else:
                        nc.gpsimd.tensor_mul(x[:], dts[:], rg[:])
                    st["x"][si] = x
                st["dt"].clear()

            def ph_erf(st, c):
                for si in range(CSQ):
                    x = st["x"][si]
                    nc.scalar.activation(x[:], x[:], AF.Erf)   # in-place

            def ph_gf(st, c):
                st["gf"] = {}
                st["g"] = {}
                for si in range(CSQ):
                    x = st["x"].pop(si)
                    rds = st["rds"].pop(si)
                    gf = gfp.tile([128, W], f32, tag="gf")
                    if SPLIT_MUL:
                        h = (int(W * POOL_FRAC) // 128) * 128
                        nc.gpsimd.tensor_mul(gf[:, 0:h], x[:, 0:h],
                                             rds[:, 0:h])
                        nc.vector.tensor_mul(gf[:, h:W], x[:, h:W],
                                             rds[:, h:W])
                    else:
                        nc.gpsimd.tensor_mul(gf[:], x[:], rds[:])
                    st["gf"][si] = gf
                    if DBG and c == 0 and si == 3:
                        nc.sync.dma_start(dbg["d_rds"][:], rds[:])
                        nc.sync.dma_start(dbg["d_ev"][:], x[:])
                        nc.sync.dma_start(dbg["d_gf"][:], gf[:])
                    g = gp.tile([128, W], bf, tag="g")
                    cp(E_GBF, g[:], gf[:])
                    st["g"][si] = g
                st["rg"].clear()

            def ph_warm(st, c):
                # sub-loops per stage so PE/ACT pipeline across superquads
                st["es"] = {}
                st["g2k"] = {}
                pbs, pb2s = {}, {}
                for si in range(CSQ):
                    g = st["g"][si]
                    pb = p_wide.tile([128, W], f32, tag="w")
                    for m in range(SQ):
                        sl = slice(m * 128, (m + 1) * 128)
                        nc.tensor.matmul(pb[:, sl], g[:, sl], g[:, sl],
                                         start=True, stop=True)
                    pbs[si] = pb
                    g2k = g2kp.tile([128, W], bf, tag="g2k")
                    if E_G2K == "a":
                        nc.scalar.activation(g2k[:], pb[:], AF.Copy,
                                             scale=SCL3)
                    else:
                        nc.vector.tensor_scalar_mul(g2k[:], pb[:], SCL3)
                    st["g2k"][si] = g2k
                for si in range(CSQ):
                    g = st["g"][si]
                    g2k = st["g2k"][si]
                    pb2 = p_wide.tile([128, W], f32, tag="w")
                    for h in range(2):     # two 512-col halves (PSUM banks)
                        hs = slice(h * 512, (h + 1) * 512)
                        nc.tensor.matmul(pb2[:, hs], k2I[:], g[:, hs],
                                         start=True, stop=False)
                        nc.tensor.matmul(pb2[:, hs], k2lo[:], g[:, hs],
                                         start=False, stop=False)
                        nc.tensor.matmul(pb2[:, hs], r23I[:], g2k[:, hs],
                                         start=False, stop=False)
                        nc.tensor.matmul(pb2[:, hs], r23lo[:], g2k[:, hs],
                                         start=False, stop=False)
                        for m in range(4 * h, 4 * h + 4):
                            sl = slice(m * 128, (m + 1) * 128)
                            nc.tensor.matmul(pb2[:, sl], g2k[:, sl],
                                             g[:, sl], start=False,
                                             stop=False)
                        nc.tensor.matmul(pb2[:, hs], identI[:],
                                         k1blk[:, hs], start=False,
                                         stop=True)
                    pb2s[si] = pb2
                    se = sep.tile([128, W], bf, tag="se")
                    cp(E_SE, se[:], pb2[:])
                    st["es"][si] = [se]
                    if DBG and c == 0 and si == 3:
                        tmp = xp.tile([128, W], f32, tag="x")
                        nc.vector.tensor_copy(tmp[:], st["g2k"][si][:])
                        nc.sync.dma_start(dbg["d_g2k"][:], tmp[:])
                        tmp2 = xp.tile([128, W], f32, tag="x")
                        nc.vector.tensor_copy(tmp2[:], se[:])
                        nc.sync.dma_start(dbg["d_se"][:], tmp2[:])

            def ph_ns(st, c, k):
                for si in range(CSQ):
                    se = st["es"][si][-1]
                    eb = p_wide.tile([128, W], f32, tag="w")
                    for m in range(SQ):
                        sl = slice(m * 128, (m + 1) * 128)
                        nc.tensor.matmul(eb[:, sl], se[:, sl], se[:, sl],
                                         start=True, stop=True)
                    se2 = sep.tile([128, W], bf, tag="se")
                    cp(ECOPY[k], se2[:], eb[:])
                    st["es"][si].append(se2)
                    if DBG and c == 0 and si == 3 and k == 0:
                        tmp = xp.tile([128, W], f32, tag="x")
                        nc.vector.tensor_copy(tmp[:], se2[:])
                        nc.sync.dma_start(dbg["d_e1"][:], tmp[:])

            def emit_apply(st, c, rhs_bf, rhs1, rhs2, w_prev):
                """w = (w_prev +) chain(rhs): X0 = b0 I + b1 G + (b2/a3) G2k,
                then K stages of (I+E_j). rhs1 = RB1*rhs, rhs2 = RB2*rhs."""
                gb = p_thin.tile([128, 2 * NM], f32, tag="t")
                for mi in range(NM):
                    si, m = mi // SQ, mi % SQ
                    sl = slice(m * 128, (m + 1) * 128)
                    ts = slice(2 * mi, 2 * mi + 2)
                    nc.tensor.matmul(gb[:, ts], b0I[:], rhs_bf[:, ts],
                                     start=True, stop=False)
                    nc.tensor.matmul(gb[:, ts], st["g"][si][:, sl],
                                     rhs1[:, ts], start=False, stop=False)
                    nc.tensor.matmul(gb[:, ts], st["g2k"][si][:, sl],
                                     rhs2[:, ts], start=False, stop=True)
                cv = cvp.tile([128, 2 * NM], bf, tag="cv")
                cp(E_CV0, cv[:], gb[:])
                for j in range(K_NS):
                    cb = p_thin.tile([128, 2 * NM], f32, tag="t")
                    for mi in range(NM):
                        si, m = mi // SQ, mi % SQ
                        sl = slice(m * 128, (m + 1) * 128)
                        nc.tensor.matmul(
                            cb[:, 2 * mi:2 * mi + 2],
                            st["es"][si][j][:, sl],
                            cv[:, 2 * mi:2 * mi + 2],
                            start=(mi == 0), stop=(mi == NM - 1))
                    if j < K_NS - 1:
                        cv2 = cvp.tile([128, 2 * NM], bf, tag="cv")
                        nc.vector.tensor_add(cv2[:], cv[:], cb[:])
                        cv = cv2
                    else:
                        w = wp.tile([128, 2 * NM], f32, tag="w")
                        if w_prev is None:
                            nc.vector.tensor_add(w[:], cv[:], cb[:])
                        else:
                            cv3 = fint.tile([128, 2 * NM], f32, tag="cv3")
                            nc.vector.tensor_add(cv3[:], cv[:], cb[:])
                            nc.vector.tensor_add(w[:], w_prev[:], cv3[:])
                return w

            def ph_fa(st, c):
                csl = slice(c * 2 * NM, (c + 1) * 2 * NM)
                st["w"] = emit_apply(st, c, btb_all[:, csl],
                                     bt1_all[:, csl], bt2_all[:, csl], None)
                if DBG and c == 0:
                    nc.sync.dma_start(dbg["d_w0"][:], st["w"][:])

            def ph_fr(st, c):
                csl = slice(c * 2 * NM, (c + 1) * 2 * NM)
                w = st["w"]
                t2 = fint.tile([128, 2 * NM], f32, tag="t2")
                nc.vector.scalar_tensor_tensor(
                    out=t2[:], in0=w[:], scalar=-C0, in1=bt_all[:, csl],
                    op0=ALU.mult, op1=ALU.add)
                pp = p_thin.tile([128, 2 * NM], f32, tag="t")
                for mi in range(NM):
                    si, m = mi // SQ, mi % SQ
                    sl = slice(m * 128, (m + 1) * 128)
                    nc.tensor.matmul(pp[:, 2 * mi:2 * mi + 2],
                                     st["gf"][si][:, sl],
                                     w[:, 2 * mi:2 * mi + 2],
                                     start=(mi == 0), stop=(mi == NM - 1))
                rt = fint.tile([128, 2 * NM], bf, tag="rt")
                nc.vector.scalar_tensor_tensor(
                    out=rt[:], in0=pp[:], scalar=-1.0, in1=t2[:],
                    op0=ALU.mult, op1=ALU.add)
                rt1 = fint.tile([128, 2 * NM], bf, tag="rt1")
                nc.vector.tensor_scalar_mul(rt1[:], rt[:], RB1)
                rt2 = fint.tile([128, 2 * NM], bf, tag="rt2")
                nc.vector.tensor_scalar_mul(rt2[:], rt[:], RB2)
                st["w"] = emit_apply(st, c, rt, rt1, rt2, w)
                if DBG and c == 0:
                    nc.sync.dma_start(dbg["d_w1"][:], st["w"][:])

            def ph_fs(st, c):
                csl = slice(c * 2 * NM, (c + 1) * 2 * NM)
                ws = lamp.tile([128, 2 * NM], f32, tag="ws")
                nc.vector.tensor_mul(ws[:], st["w"][:], shS_all[:, csl])
                sums = p_thin.tile([1, 2 * NM], f32, tag="t")
                nc.tensor.matmul(sums[:], ones_col[:], ws[:])
                num = lamp.tile([1, NM], f32, tag="num")
                nc.vector.tensor_add(
                    num[:], sums[0:1, 0:2 * NM:2],
                    q_all[:, c * NM:(c + 1) * NM])
                den = lamp.tile([1, NM], f32, tag="den")
                nc.vector.tensor_scalar_add(den[:], sums[0:1, 1:2 * NM:2],
                                            -1.0)
                rden = lamp.tile([1, NM], f32, tag="rden")
                nc.vector.reciprocal(rden[:], den[:])
                lamneg = lamp.tile([1, NM], f32, tag="lamneg")
                nc.vector.tensor_mul(lamneg[:], num[:], rden[:])
                lamb = lamp.tile([128, NM], f32, tag="lamb")
                nc.gpsimd.partition_broadcast(lamb[:], lamneg[:])
                t1 = lamp.tile([128, NM], f32, tag="t1")
                nc.vector.tensor_mul(t1[:], ws[:, 1:2 * NM:2], lamb[:])
                qc = lamp.tile([128, NM], f32, tag="qc")
                nc.vector.tensor_sub(qc[:], t1[:], ws[:, 0:2 * NM:2])
                nc.sync.dma_start(out[:, c * NM:(c + 1) * NM], qc[:])
                st["es"].clear()
                st["gf"].clear()
                st["g"].clear()
                st["g2k"].clear()

            # phase table (per-cohort refinement count)
            rlist = (R_LIST if len(R_LIST) == NCOH
                     else [N_REF] * NCOH)

            def emit_phase(st, c, ph):
                rc = rlist[c]
                if ph == 0:
                    ph_build(st, c)
                elif ph == 1:
                    ph_sqrt(st, c)
                elif ph == 2:
                    ph_rx(st, c)
                elif ph == 3:
                    ph_erf(st, c)
                elif ph == 4:
                    ph_gf(st, c)
                elif ph == 5:
                    ph_warm(st, c)
                elif ph < 5 + K_NS:
                    ph_ns(st, c, ph - 6)
                elif ph == 5 + K_NS:
                    ph_fa(st, c)
                elif ph < 6 + K_NS + rc:
                    ph_fr(st, c)
                elif ph == 6 + K_NS + rc:
                    ph_fs(st, c)

            NPH = 7 + K_NS + max(R_LIST + [N_REF])
            states = [dict() for _ in range(NCOH)]
            total = OFF * (NCOH - 1) + NPH
            for t in range(total):
                for c in range(NCOH):
                    ph = t - OFF * c
                    if 0 <= ph < NPH:
                        emit_phase(states[c], c, ph)

    nc.compile()
    return nc


def _host_pack(eneg, positions, node_attrs, hardness, total_charge,
               atomic_numbers):
    """Precompute per-atom quantities and pack per-core DRAM tensors."""
    f32 = np.float32
    pos = np.ascontiguousarray(positions, dtype=f32).reshape(B_MOL, N_ATOM, 3)
    Z = np.asarray(atomic_numbers).astype(np.int64).reshape(B_MOL, N_ATOM)
    na = np.asarray(node_attrs, dtype=f32).reshape(B_MOL, N_ATOM, -1)
    hard = np.asarray(hardness, dtype=f32)
    e = np.asarray(eneg, dtype=f32).reshape(B_MOL, N_ATOM)
    Q = np.asarray(total_charge, dtype=f32).reshape(B_MOL)

    cov = (0.3 + 0.02 * np.arange(100)).astype(f32)
    r = cov[Z]                                   # [B, n]
    sig = (r * r).astype(f32)
    n2 = (pos * pos).sum(axis=2, dtype=f32).astype(f32)
    aidx = na.argmax(axis=2)
    dv = (hard[aidx] + f32(1.0) / (np.sqrt(np.pi).astype(f32) * r)).astype(f32)
    sh = (f32(1.0) / np.sqrt(dv)).astype(f32)    # s = 1/sqrt(diag A)

    def to_fp16(x):
        return np.ascontiguousarray(
            np.asarray(x, dtype=np.float32).astype(np.float16))

    from scipy.special import erf as _erf

    # difficulty proxy: max scaled offdiag Gershgorin row sum per molecule
    prox = np.empty(B_MOL, dtype=np.float64)
    for c in range(N_CORES):
        sl = slice(c * MPC, (c + 1) * MPC)
        p = pos[sl]
        diff = p[:, :, None, :] - p[:, None, :, :]
        dd2 = (diff * diff).sum(-1) + np.eye(N_ATOM, dtype=f32)
        dist = np.sqrt(dd2)
        sgl = sig[sl]
        gam2 = 2.0 * (sgl[:, :, None] + sgl[:, None, :])
        shl = sh[sl]
        Aoff = (_erf(dist / np.sqrt(gam2)) / dist
                * np.einsum("mi,mj->mij", shl, shl))
        ii = np.arange(N_ATOM)
        Aoff[:, ii, ii] = 0.0
        prox[sl] = np.abs(Aoff).sum(2).max(1)

    mpc = MPC
    npair = mpc // 2
    in_maps = []
    perms = []
    for c in range(N_CORES):
        sl = slice(c * mpc, (c + 1) * mpc)
        perm = np.argsort(-prox[sl], kind="stable")   # hardest first
        perms.append(perm)
        p = pos[sl][perm]    # [mpc, 128, 3]
        nn2 = n2[sl][perm]
        sgl = sig[sl][perm]
        shl = sh[sl][perm]   # [mpc, 128]
        el = e[sl][perm]

        F = (f32(S0) / (shl * shl)).astype(f32)       # S0/s^2  [mpc, n]
        # per-molecule scaled lhs rows [5, n] and rhs rows [5, n]
        lhs5 = np.stack([-2.0 * p[:, :, 0] * F, -2.0 * p[:, :, 1] * F,
                         -2.0 * p[:, :, 2] * F, (nn2 + EPS_D2) * F, F],
                        axis=1).astype(f32)            # [mpc, 5, n]
        rhs5 = np.stack([p[:, :, 0] * F, p[:, :, 1] * F, p[:, :, 2] * F,
                         F, nn2 * F], axis=1).astype(f32)

        lhsp = np.zeros((10, npair, N_ATOM), dtype=f32)
        lhsp[0:5] = lhs5[0::2].transpose(1, 0, 2)
        lhsp[5:10] = lhs5[1::2].transpose(1, 0, 2)
        rhsp = np.zeros((10, npair, 2 * N_ATOM), dtype=f32)
        rhsp[0:5, :, :N_ATOM] = rhs5[0::2].transpose(1, 0, 2)
        rhsp[5:10, :, N_ATOM:] = rhs5[1::2].transpose(1, 0, 2)

        # rgs = s_i s_j / (S0 * sqrt(2 sig_i + 2 sig_j)), diag 0
        gam2 = 2.0 * (sgl[:, :, None] + sgl[:, None, :])
        rgsp = (np.einsum("mi,mj->mij", shl, shl)
                / (f32(S0) * np.sqrt(gam2))).astype(f32)
        ii = np.arange(N_ATOM)
        rgsp[:, ii, ii] = 0.0
        rgsp = np.ascontiguousarray(
            rgsp.transpose(1, 0, 2).reshape(N_ATOM, mpc * N_ATOM))

        btpk = np.empty((N_ATOM, 2 * mpc), dtype=f32)
        btpk[:, 0::2] = (el * shl / f32(S0)).T
        btpk[:, 1::2] = (shl / f32(S0)).T
        shSp = np.empty((N_ATOM, 2 * mpc), dtype=f32)
        shSp[:, 0::2] = shl.T
        shSp[:, 1::2] = shl.T
        qp = np.ascontiguousarray(Q[sl][perm]).reshape(1, mpc)
        in_maps.append({
            "lhsr_pack": np.ascontiguousarray(
                lhsp.reshape(10, npair * N_ATOM)),
            "rhsr_pack": np.ascontiguousarray(
                rhsp.reshape(10, npair * 2 * N_ATOM)),
            "rgs_pack": rgsp,
            "bt_pack": btpk,
            "bt1_pack": to_fp16(btpk * f32(RB1)),
            "bt2_pack": to_fp16(btpk * f32(RB2)),
            "shS_pack": shSp,
            "q_pack": qp,
        })
    return in_maps, perms


def run_device(in_maps, trace=False, **kw):
    if "nc" not in _CACHE:
        _CACHE["nc"] = _build_bass()
    nc = _CACHE["nc"]
    return run_bass_kernel_spmd(nc, in_maps, list(range(N_CORES)),
                                trace=trace, **kw)


def kernel(eneg, positions, node_attrs, hardness, total_charge, batch,
           atomic_numbers):
    in_maps, perms = _host_pack(eneg, positions, node_attrs, hardness,
                                total_charge, atomic_numbers)
    res = run_device(in_maps)
    outs = []
    for c in range(N_CORES):
        o = np.ascontiguousarray(res.results[c]["out"].T)   # [mol, atom]
        restored = np.empty_like(o)
        restored[perms[c]] = o
        outs.append(restored)
    full = np.concatenate(outs, axis=0).reshape(-1).astype(np.float32)
    return full


# revision 21
# speedup vs baseline: 1.2263x; 1.0014x over previous
"""Charge-equilibration kernel for Trainium2 (8 NeuronCores, SPMD) — v2.

Problem: 1024 molecules x 128 atoms. Per molecule build the erf-screened
Coulomb matrix A, solve the augmented system via Schur complement, return
partial charges [131072] f32.

v2 algorithm (per core: 128 molecules, data-parallel across cores):
  - Jacobi-scaled system At = D_s A D_s (unit diagonal), M = At/S0.
  - Host packs scaled build tensors so the device computes
      d~^2 = d^2 * S0^2/(s_i s_j)^2  (PE, paired-molecule f32r matmuls)
      dt  = sqrt(d~^2)               (ACT)
      rds = 1/dt = s_i s_j/(S0 d)    (DVE reciprocal)
      x   = dt * rgs                 (Pool; rgs = s_i s_j/(S0*sqrt(2)gam),
                                      f32 host pack, diag 0 -> Gf diag 0)
      ev  = erf(x)                   (ACT)
      Gf  = ev * rds                 (Pool; f32 == offdiag(At)/S0)
      g   = bf16(Gf)
  - Cubic Chebyshev seed on [a,b]:
      E0 = a0 I + a1 G + a2 G^2 + a3 G^3,  X0 = b0 I + b1 G + b2 G^2
    built via PE PSUM accumulation (G2k = bf16(a3*G^2) via ACT scale-copy).
  - K-1 product-form NS squarings E_{j+1} = E_j^2 (bf16), chain applies
    to thin rhs ([128, 2/molecule]) + R refinements against exact f32 Gf.
  - Schur: lam = (Q + sum v)/(1 - sum u), q = -(v + lam*u).

Emission is software-pipelined over cohorts of 4 superquads (8 molecules
each, [128,1024] tiles) with phase-batched ACT table usage.
"""

import os
import numpy as np

import concourse.bass as bass
import concourse.bacc as bacc
import concourse.tile as tile
import concourse.mybir as mybir
from concourse.bass_utils import run_bass_kernel_spmd
from concourse.masks import make_identity

dt = mybir.dt
AF = mybir.ActivationFunctionType
ALU = mybir.AluOpType

N_CORES = 8
B_MOL = 1024
N_ATOM = 128
MPC = B_MOL // N_CORES          # molecules per core = 128
SQ = 8                          # molecules per superquad
NSQ = MPC // SQ                 # 16 superquads
CSQ = int(os.environ.get("KE_CSQ", "4"))  # superquads per cohort
NCOH = NSQ // CSQ               # 4 cohorts
NM = CSQ * SQ                   # molecules per cohort = 32
W = SQ * N_ATOM                 # superquad tile width = 1024

S0 = float(os.environ.get("KE_S0", "32.5"))
CH_A = float(os.environ.get("KE_A", "0.0425"))
CH_B = float(os.environ.get("KE_B", "34.5"))
K_NS = int(os.environ.get("KE_K", "6"))     # chain length (K-1 squarings)
N_REF = int(os.environ.get("KE_R", "2"))
R_LIST = [int(x) for x in os.environ.get("KE_RL", "2,2,2,1").split(",")]
OFF = int(os.environ.get("KE_OFF", "12"))   # cohort pipeline offset (ticks)
EPS_D2 = 1.0e-4
L_DIAG = 1.0e10
SQRT_L = float(np.sqrt(L_DIAG))
C0 = 1.0 / S0

# engine knobs: a=ACT, v=DVE, p=Pool
E_SE = os.environ.get("KE_SE", "v")             # E0 evacuation
ECOPY = os.environ.get("KE_ECOPY", "avava")     # squaring evacs (K_NS-1)
E_GBF = os.environ.get("KE_GBF", "v")           # g = bf16(Gf)
E_CV0 = os.environ.get("KE_CV0", "v")           # cv0 thin evac
SPLIT_MUL = os.environ.get("KE_SPLIT", "1") == "1"
E_G2K = os.environ.get("KE_G2K", "a")           # G2k evac engine
POOL_FRAC = float(os.environ.get("KE_PF", "0.5625"))  # pool share of split muls
BUILD_F32R = os.environ.get("KE_BF32R", "0") == "1"


def _cheb_seed(a, b, deg, S0v):
    import numpy.polynomial.chebyshev as C
    import numpy.polynomial.polynomial as P
    from math import comb
    cheb = np.zeros(deg + 1)
    cheb[deg] = 1.0
    tpoly = C.cheb2poly(cheb)
    u0 = (b + a) / (b - a)
    u1 = -2.0 / (b - a)
    t_lam = np.zeros(deg + 1)
    for k in range(deg + 1):
        if tpoly[k] == 0.0:
            continue
        binom = P.polypow([u0, u1], k) if k > 0 else np.array([1.0])
        t_lam[:len(binom)] += tpoly[k] * binom
    t0 = np.polynomial.chebyshev.chebval(u0, cheb)
    e_lam = t_lam / t0
    x_lam = -e_lam[1:]
    alpha = np.zeros(deg + 1)
    beta = np.zeros(deg)
    c0 = 1.0 / S0v
    for k in range(deg + 1):
        ck = e_lam[k] * S0v ** k
        for j in range(k + 1):
            alpha[j] += ck * comb(k, j) * c0 ** (k - j)
    for k in range(deg):
        ck = x_lam[k] * S0v ** k
        for j in range(k + 1):
            beta[j] += ck * comb(k, j) * c0 ** (k - j)
    beta *= S0v
    return alpha, beta


ALPHA, BETA = _cheb_seed(CH_A, CH_B, 3, S0)
SCL3 = float(ALPHA[3])          # exact f32 scale used in G2k ACT copy
RB1 = float(BETA[1])            # rhs1 scale
RB2 = float(BETA[2] / SCL3)     # rhs2 scale (G2k-term)


def _bf_split(v):
    hi = float(np.float32(v).astype(np.float16).astype(np.float32))
    lo = float(np.float32(v - hi))
    return hi, lo


A1_HI, A1_LO = _bf_split(ALPHA[1])
R23_HI, R23_LO = _bf_split(ALPHA[2] / SCL3)

_CACHE = {}


def _build_bass():
    nc = bacc.Bacc()
    f32 = dt.float32
    bf = dt.float16
    bfL = dt.bfloat16

    NPAIR = MPC // 2
    lhs = nc.declare_dram_parameter("lhsr_pack", [10, NPAIR * N_ATOM], f32,
                                    isOutput=False)
    rhs = nc.declare_dram_parameter("rhsr_pack", [10, NPAIR * 2 * N_ATOM],
                                    f32, isOutput=False)
    rgs = nc.declare_dram_parameter("rgs_pack", [N_ATOM, MPC * N_ATOM], f32,
                                    isOutput=False)
    btp = nc.declare_dram_parameter("bt_pack", [N_ATOM, 2 * MPC], f32,
                                    isOutput=False)
    bt1p = nc.declare_dram_parameter("bt1_pack", [N_ATOM, 2 * MPC], bf,
                                     isOutput=False)
    bt2p = nc.declare_dram_parameter("bt2_pack", [N_ATOM, 2 * MPC], bf,
                                     isOutput=False)
    shS = nc.declare_dram_parameter("shS_pack", [N_ATOM, 2 * MPC], f32,
                                    isOutput=False)
    qpk = nc.declare_dram_parameter("q_pack", [1, MPC], f32, isOutput=False)
    out = nc.declare_dram_parameter("out", [N_ATOM, MPC], f32, isOutput=True)
    DBG = os.environ.get("KE_DBG", "") == "1"
    dbg = {}
    if DBG:
        for nm, w_ in [("d_dt", W), ("d_rds", W), ("d_ev", W), ("d_gf", W),
                       ("d_g2k", W), ("d_se", W), ("d_e1", W), ("d_w0", 64),
                       ("d_w1", 64)]:
            dbg[nm] = nc.declare_dram_parameter(nm, [N_ATOM, w_], f32,
                                                isOutput=True)

    def cp(eng, dst, src):
        if eng == "a":
            nc.scalar.copy(dst, src)
        elif eng == "v":
            nc.vector.tensor_copy(dst, src)
        else:
            nc.gpsimd.tensor_copy(dst, src)

    from contextlib import ExitStack

    with tile.TileContext(nc) as tc:
        with ExitStack() as es:
            def pool(name, bufs, space=None):
                kw = {"space": space} if space else {}
                return es.enter_context(
                    tc.tile_pool(name=name, bufs=bufs, **kw))

            const = pool("const", 1)
            core_in = pool("core_in", 1)
            DP = int(os.environ.get("KE_DP", "3"))
            lhs_in = pool("lhs_in", DP)
            rhs_in = pool("rhs_in", DP)
            rgs_in = pool("rgs_in", DP + 1)
            dtp = pool("dtp", DP)
            rdsp = pool("rdsp", DP)
            xp = pool("xp", DP)
            gfp = pool("gfp", CSQ + 2)
            gp = pool("gp", CSQ + 2)
            g2kp = pool("g2kp", CSQ + 2)
            sep = pool("sep", K_NS * CSQ + 4)
            cvp = pool("cvp", 4)
            wp = pool("wp", 5)
            fint = pool("fint", 8)
            lamp = pool("lamp", 8)
            p_wide = pool("p_wide", 3, "PSUM")
            p_thin = pool("p_thin", 2, "PSUM")

            # ---- constants ----
            identI = const.tile([128, 128], bf)
            make_identity(nc, identI[:])          # plain identity (for k1blk)
            k2I = const.tile([128, 128], bf)
            nc.gpsimd.memset(k2I[:], A1_HI)
            nc.gpsimd.affine_select(
                out=k2I[:], in_=k2I[:], compare_op=ALU.is_equal,
                fill=0.0, base=0, pattern=[[-1, 128]], channel_multiplier=1)
            r23I = const.tile([128, 128], bf)
            nc.gpsimd.memset(r23I[:], R23_HI)
            nc.gpsimd.affine_select(
                out=r23I[:], in_=r23I[:], compare_op=ALU.is_equal,
                fill=0.0, base=0, pattern=[[-1, 128]], channel_multiplier=1)
            k2lo = const.tile([128, 128], bf)
            nc.gpsimd.memset(k2lo[:], A1_LO)
            nc.gpsimd.affine_select(
                out=k2lo[:], in_=k2lo[:], compare_op=ALU.is_equal,
                fill=0.0, base=0, pattern=[[-1, 128]], channel_multiplier=1)
            r23lo = const.tile([128, 128], bf)
            nc.gpsimd.memset(r23lo[:], R23_LO)
            nc.gpsimd.affine_select(
                out=r23lo[:], in_=r23lo[:], compare_op=ALU.is_equal,
                fill=0.0, base=0, pattern=[[-1, 128]], channel_multiplier=1)
            b0I = const.tile([128, 128], bf)
            nc.gpsimd.memset(b0I[:], float(BETA[0]))
            nc.gpsimd.affine_select(
                out=b0I[:], in_=b0I[:], compare_op=ALU.is_equal,
                fill=0.0, base=0, pattern=[[-1, 128]], channel_multiplier=1)
            k1blk = const.tile([128, W], bf)
            nc.gpsimd.memset(k1blk[:], float(ALPHA[0]))
            nc.gpsimd.affine_select(
                out=k1blk[:], in_=k1blk[:], compare_op=ALU.is_equal,
                fill=0.0, base=0, pattern=[[0, SQ], [-1, 128]],
                channel_multiplier=1)
            ones_col = const.tile([128, 1], f32)
            nc.gpsimd.memset(ones_col[:], 1.0)

            # ---- whole-core small inputs ----
            bt_all = core_in.tile([N_ATOM, 2 * MPC], f32)
            nc.sync.dma_start(bt_all[:], btp[:])
            bt1_all = core_in.tile([N_ATOM, 2 * MPC], bf)
            nc.sync.dma_start(bt1_all[:], bt1p[:])
            bt2_all = core_in.tile([N_ATOM, 2 * MPC], bf)
            nc.sync.dma_start(bt2_all[:], bt2p[:])
            shS_all = core_in.tile([N_ATOM, 2 * MPC], f32)
            nc.sync.dma_start(shS_all[:], shS[:])
            q_all = core_in.tile([1, MPC], f32)
            nc.sync.dma_start(q_all[:], qpk[:])
            btb_all = core_in.tile([N_ATOM, 2 * MPC], bf)
            nc.vector.tensor_copy(btb_all[:], bt_all[:])

            # ---- phases (cohort state dicts) ----
            def ph_build(st, c):
                st["pd"] = {}
                st["dt"] = {}
                for si in range(CSQ):
                    sq = c * CSQ + si
                    lh = lhs_in.tile([10, 4 * 128], f32, tag="lh")
                    nc.sync.dma_start(
                        lh[:], lhs[:, sq * 4 * 128:(sq + 1) * 4 * 128])
                    rh = rhs_in.tile([10, 4 * 256], f32, tag="rh")
                    nc.sync.dma_start(
                        rh[:], rhs[:, sq * 4 * 256:(sq + 1) * 4 * 256])
                    rg = rgs_in.tile([128, W], f32, tag="rg")
                    nc.sync.dma_start(
                        rg[:], rgs[:, sq * W:(sq + 1) * W])
                    st.setdefault("rg", {})[si] = rg
                    pd = p_wide.tile([128, W], f32, tag="w")
                    for p in range(4):      # pairs of molecules
                        lsl = lh[:, p * 128:(p + 1) * 128]
                        rsl = rh[:, p * 256:(p + 1) * 256]
                        if BUILD_F32R:
                            lsl = lsl.bitcast(dt.float32r)
                            rsl = rsl.bitcast(dt.float32r)
                        ps = slice(p * 256, (p + 1) * 256)
                        nc.tensor.matmul(pd[:, ps], lsl, rsl,
                                         start=True, stop=True)
                    st["pd"][si] = pd

            def ph_sqrt(st, c):
                for si in range(CSQ):
                    pd = st["pd"].pop(si)
                    dts = dtp.tile([128, W], f32, tag="dt")
                    nc.scalar.sqrt(dts[:], pd[:])
                    st["dt"][si] = dts
                    if DBG and c == 0 and si == 3:
                        nc.sync.dma_start(dbg["d_dt"][:], dts[:])

            def ph_rx(st, c):
                st["rds"] = {}
                st["x"] = {}
                for si in range(CSQ):
                    dts = st["dt"][si]
                    rds = rdsp.tile([128, W], f32, tag="rds")
                    nc.vector.reciprocal(rds[:], dts[:])
                    st["rds"][si] = rds
                    x = xp.tile([128, W], f32, tag="x")
                    rg = st["rg"][si]
                    if SPLIT_MUL:
                        h = (int(W * POOL_FRAC) // 128) * 128
                        nc.gpsimd.tensor_mul(x[:, 0:h], dts[:, 0:h],
                                             rg[:, 0:h])
                        nc.vector.tensor_mul(x[:, h:W], dts[:, h:W],
                                             rg[:, h:W])
                    else:
                        nc.gpsimd.tensor_mul(x[:], dts[:], rg[:])
                    st["x"][si] = x
                st["dt"].clear()

            def ph_erf(st, c):
                for si in range(CSQ):
                    x = st["x"][si]
                    nc.scalar.activation(x[:], x[:], AF.Erf)   # in-place

            def ph_gf(st, c):
                st["gf"] = {}
                st["g"] = {}
                for si in range(CSQ):
                    x = st["x"].pop(si)
                    rds = st["rds"].pop(si)
                    gf = gfp.tile([128, W], f32, tag="gf")
                    if SPLIT_MUL:
                        h = (int(W * POOL_FRAC) // 128) * 128
                        nc.gpsimd.tensor_mul(gf[:, 0:h], x[:, 0:h],
                                             rds[:, 0:h])
                        nc.vector.tensor_mul(gf[:, h:W], x[:, h:W],
                                             rds[:, h:W])
                    else:
                        nc.gpsimd.tensor_mul(gf[:], x[:], rds[:])
                    st["gf"][si] = gf
                    if DBG and c == 0 and si == 3:
                        nc.sync.dma_start(dbg["d_rds"][:], rds[:])
                        nc.sync.dma_start(dbg["d_ev"][:], x[:])
                        nc.sync.dma_start(dbg["d_gf"][:], gf[:])
                    g = gp.tile([128, W], bf, tag="g")
                    cp(E_GBF, g[:], gf[:])
                    st["g"][si] = g
                st["rg"].clear()

            def ph_warm(st, c):
                # sub-loops per stage so PE/ACT pipeline across superquads
                st["es"] = {}
                st["g2k"] = {}
                pbs, pb2s = {}, {}
                for si in range(CSQ):
                    g = st["g"][si]
                    pb = p_wide.tile([128, W], f32, tag="w")
                    for m in range(SQ):
                        sl = slice(m * 128, (m + 1) * 128)
                        nc.tensor.matmul(pb[:, sl], g[:, sl], g[:, sl],
                                         start=True, stop=True)
                    pbs[si] = pb
                    g2k = g2kp.tile([128, W], bf, tag="g2k")
                    if E_G2K == "a":
                        nc.scalar.activation(g2k[:], pb[:], AF.Copy,
                                             scale=SCL3)
                    else:
                        nc.vector.tensor_scalar_mul(g2k[:], pb[:], SCL3)
                    st["g2k"][si] = g2k
                for si in range(CSQ):
                    g = st["g"][si]
                    g2k = st["g2k"][si]
                    pb2 = p_wide.tile([128, W], f32, tag="w")
                    for h in range(2):     # two 512-col halves (PSUM banks)
                        hs = slice(h * 512, (h + 1) * 512)
                        nc.tensor.matmul(pb2[:, hs], k2I[:], g[:, hs],
                                         start=True, stop=False)
                        nc.tensor.matmul(pb2[:, hs], k2lo[:], g[:, hs],
                                         start=False, stop=False)
                        nc.tensor.matmul(pb2[:, hs], r23I[:], g2k[:, hs],
                                         start=False, stop=False)
                        nc.tensor.matmul(pb2[:, hs], r23lo[:], g2k[:, hs],
                                         start=False, stop=False)
                        for m in range(4 * h, 4 * h + 4):
                            sl = slice(m * 128, (m + 1) * 128)
                            nc.tensor.matmul(pb2[:, sl], g2k[:, sl],
                                             g[:, sl], start=False,
                                             stop=False)
                        nc.tensor.matmul(pb2[:, hs], identI[:],
                                         k1blk[:, hs], start=False,
                                         stop=True)
                    pb2s[si] = pb2
                    se = sep.tile([128, W], bf, tag="se")
                    cp(E_SE, se[:], pb2[:])
                    st["es"][si] = [se]
                    if DBG and c == 0 and si == 3:
                        tmp = xp.tile([128, W], f32, tag="x")
                        nc.vector.tensor_copy(tmp[:], st["g2k"][si][:])
                        nc.sync.dma_start(dbg["d_g2k"][:], tmp[:])
                        tmp2 = xp.tile([128, W], f32, tag="x")
                        nc.vector.tensor_copy(tmp2[:], se[:])
                        nc.sync.dma_start(dbg["d_se"][:], tmp2[:])

            def ph_ns(st, c, k):
                for si in range(CSQ):
                    se = st["es"][si][-1]
                    eb = p_wide.tile([128, W], f32, tag="w")
                    for m in range(SQ):
                        sl = slice(m * 128, (m + 1) * 128)
                        nc.tensor.matmul(eb[:, sl], se[:, sl], se[:, sl],
                                         start=True, stop=True)
                    se2 = sep.tile([128, W], bf, tag="se")
                    cp(ECOPY[k], se2[:], eb[:])
                    st["es"][si].append(se2)
                    if DBG and c == 0 and si == 3 and k == 0:
                        tmp = xp.tile([128, W], f32, tag="x")
                        nc.vector.tensor_copy(tmp[:], se2[:])
                        nc.sync.dma_start(dbg["d_e1"][:], tmp[:])

            def emit_apply(st, c, rhs_bf, rhs1, rhs2, w_prev):
                """w = (w_prev +) chain(rhs): X0 = b0 I + b1 G + (b2/a3) G2k,
                then K stages of (I+E_j). rhs1 = RB1*rhs, rhs2 = RB2*rhs."""
                gb = p_thin.tile([128, 2 * NM], f32, tag="t")
                for mi in range(NM):
                    si, m = mi // SQ, mi % SQ
                    sl = slice(m * 128, (m + 1) * 128)
                    ts = slice(2 * mi, 2 * mi + 2)
                    nc.tensor.matmul(gb[:, ts], b0I[:], rhs_bf[:, ts],
                                     start=True, stop=False)
                    nc.tensor.matmul(gb[:, ts], st["g"][si][:, sl],
                                     rhs1[:, ts], start=False, stop=False)
                    nc.tensor.matmul(gb[:, ts], st["g2k"][si][:, sl],
                                     rhs2[:, ts], start=False, stop=True)
                cv = cvp.tile([128, 2 * NM], bf, tag="cv")
                cp(E_CV0, cv[:], gb[:])
                for j in range(K_NS):
                    cb = p_thin.tile([128, 2 * NM], f32, tag="t")
                    for mi in range(NM):
                        si, m = mi // SQ, mi % SQ
                        sl = slice(m * 128, (m + 1) * 128)
                        nc.tensor.matmul(
                            cb[:, 2 * mi:2 * mi + 2],
                            st["es"][si][j][:, sl],
                            cv[:, 2 * mi:2 * mi + 2],
                            start=(mi == 0), stop=(mi == NM - 1))
                    if j < K_NS - 1:
                        cv2 = cvp.tile([128, 2 * NM], bf, tag="cv")
                        nc.vector.tensor_add(cv2[:], cv[:], cb[:])
                        cv = cv2
                    else:
                        w = wp.tile([128, 2 * NM], f32, tag="w")
                        if w_prev is None:
                            nc.vector.tensor_add(w[:], cv[:], cb[:])
                        else:
                            cv3 = fint.tile([128, 2 * NM], f32, tag="cv3")
                            nc.vector.tensor_add(cv3[:], cv[:], cb[:])
                            nc.vector.tensor_add(w[:], w_prev[:], cv3[:])
                return w

            def ph_fa(st, c):
                csl = slice(c * 2 * NM, (c + 1) * 2 * NM)
                st["w"] = emit_apply(st, c, btb_all[:, csl],
                                     bt1_all[:, csl], bt2_all[:, csl], None)
                if DBG and c == 0:
                    nc.sync.dma_start(dbg["d_w0"][:], st["w"][:])

            def ph_fr(st, c):
                csl = slice(c * 2 * NM, (c + 1) * 2 * NM)
                w = st["w"]
                t2 = fint.tile([128, 2 * NM], f32, tag="t2")
                nc.vector.scalar_tensor_tensor(
                    out=t2[:], in0=w[:], scalar=-C0, in1=bt_all[:, csl],
                    op0=ALU.mult, op1=ALU.add)
                pp = p_thin.tile([128, 2 * NM], f32, tag="t")
                for mi in range(NM):
                    si, m = mi // SQ, mi % SQ
                    sl = slice(m * 128, (m + 1) * 128)
                    nc.tensor.matmul(pp[:, 2 * mi:2 * mi + 2],
                                     st["gf"][si][:, sl],
                                     w[:, 2 * mi:2 * mi + 2],
                                     start=(mi == 0), stop=(mi == NM - 1))
                rt = fint.tile([128, 2 * NM], bf, tag="rt")
                nc.vector.scalar_tensor_tensor(
                    out=rt[:], in0=pp[:], scalar=-1.0, in1=t2[:],
                    op0=ALU.mult, op1=ALU.add)
                rt1 = fint.tile([128, 2 * NM], bf, tag="rt1")
                nc.vector.tensor_scalar_mul(rt1[:], rt[:], RB1)
                rt2 = fint.tile([128, 2 * NM], bf, tag="rt2")
                nc.vector.tensor_scalar_mul(rt2[:], rt[:], RB2)
                st["w"] = emit_apply(st, c, rt, rt1, rt2, w)
                if DBG and c == 0:
                    nc.sync.dma_start(dbg["d_w1"][:], st["w"][:])

            def ph_fs(st, c):
                csl = slice(c * 2 * NM, (c + 1) * 2 * NM)
                ws = lamp.tile([128, 2 * NM], f32, tag="ws")
                nc.vector.tensor_mul(ws[:], st["w"][:], shS_all[:, csl])
                sums = p_thin.tile([1, 2 * NM], f32, tag="t")
                nc.tensor.matmul(sums[:], ones_col[:], ws[:])
                num = lamp.tile([1, NM], f32, tag="num")
                nc.vector.tensor_add(
                    num[:], sums[0:1, 0:2 * NM:2],
                    q_all[:, c * NM:(c + 1) * NM])
                den = lamp.tile([1, NM], f32, tag="den")
                nc.vector.tensor_scalar_add(den[:], sums[0:1, 1:2 * NM:2],
                                            -1.0)
                rden = lamp.tile([1, NM], f32, tag="rden")
                nc.vector.reciprocal(rden[:], den[:])
                lamneg = lamp.tile([1, NM], f32, tag="lamneg")
                nc.vector.tensor_mul(lamneg[:], num[:], rden[:])
                lamb = lamp.tile([128, NM], f32, tag="lamb")
                nc.gpsimd.partition_broadcast(lamb[:], lamneg[:])
                t1 = lamp.tile([128, NM], f32, tag="t1")
                nc.vector.tensor_mul(t1[:], ws[:, 1:2 * NM:2], lamb[:])
                qc = lamp.tile([128, NM], f32, tag="qc")
                nc.vector.tensor_sub(qc[:], t1[:], ws[:, 0:2 * NM:2])
                nc.sync.dma_start(out[:, c * NM:(c + 1) * NM], qc[:])
                st["es"].clear()
                st["gf"].clear()
                st["g"].clear()
                st["g2k"].clear()

            # phase table (per-cohort refinement count)
            rlist = (R_LIST if len(R_LIST) == NCOH
                     else [N_REF] * NCOH)

            def emit_phase(st, c, ph):
                rc = rlist[c]
                if ph == 0:
                    ph_build(st, c)
                elif ph == 1:
                    ph_sqrt(st, c)
                elif ph == 2:
                    ph_rx(st, c)
                elif ph == 3:
                    ph_erf(st, c)
                elif ph == 4:
                    ph_gf(st, c)
                elif ph == 5:
                    ph_warm(st, c)
                elif ph < 5 + K_NS:
                    ph_ns(st, c, ph - 6)
                elif ph == 5 + K_NS:
                    ph_fa(st, c)
                elif ph < 6 + K_NS + rc:
                    ph_fr(st, c)
                elif ph == 6 + K_NS + rc:
                    ph_fs(st, c)

            NPH = 7 + K_NS + max(R_LIST + [N_REF])
            states = [dict() for _ in range(NCOH)]
            total = OFF * (NCOH - 1) + NPH
            for t in range(total):
                for c in range(NCOH):
                    ph = t - OFF * c
                    if 0 <= ph < NPH:
                        emit_phase(states[c], c, ph)

    nc.compile()
    return nc


def _host_pack(eneg, positions, node_attrs, hardness, total_charge,
               atomic_numbers):
    """Precompute per-atom quantities and pack per-core DRAM tensors."""
    f32 = np.float32
    pos = np.ascontiguousarray(positions, dtype=f32).reshape(B_MOL, N_ATOM, 3)
    Z = np.asarray(atomic_numbers).astype(np.int64).reshape(B_MOL, N_ATOM)
    na = np.asarray(node_attrs, dtype=f32).reshape(B_MOL, N_ATOM, -1)
    hard = np.asarray(hardness, dtype=f32)
    e = np.asarray(eneg, dtype=f32).reshape(B_MOL, N_ATOM)
    Q = np.asarray(total_charge, dtype=f32).reshape(B_MOL)

    cov = (0.3 + 0.02 * np.arange(100)).astype(f32)
    r = cov[Z]                                   # [B, n]
    sig = (r * r).astype(f32)
    n2 = (pos * pos).sum(axis=2, dtype=f32).astype(f32)
    aidx = na.argmax(axis=2)
    dv = (hard[aidx] + f32(1.0) / (np.sqrt(np.pi).astype(f32) * r)).astype(f32)
    sh = (f32(1.0) / np.sqrt(dv)).astype(f32)    # s = 1/sqrt(diag A)

    def to_fp16(x):
        return np.ascontiguousarray(
            np.asarray(x, dtype=np.float32).astype(np.float16))

    from scipy.special import erf as _erf

    # difficulty proxy: max scaled offdiag Gershgorin row sum per molecule
    prox = np.empty(B_MOL, dtype=np.float64)
    for c in range(N_CORES):
        sl = slice(c * MPC, (c + 1) * MPC)
        p = pos[sl]
        diff = p[:, :, None, :] - p[:, None, :, :]
        dd2 = (diff * diff).sum(-1) + np.eye(N_ATOM, dtype=f32)
        dist = np.sqrt(dd2)
        sgl = sig[sl]
        gam2 = 2.0 * (sgl[:, :, None] + sgl[:, None, :])
        shl = sh[sl]
        Aoff = (_erf(dist / np.sqrt(gam2)) / dist
                * np.einsum("mi,mj->mij", shl, shl))
        ii = np.arange(N_ATOM)
        Aoff[:, ii, ii] = 0.0
        prox[sl] = np.abs(Aoff).sum(2).max(1)

    mpc = MPC
    npair = mpc // 2
    in_maps = []
    perms = []
    for c in range(N_CORES):
        sl = slice(c * mpc, (c + 1) * mpc)
        perm = np.argsort(-prox[sl], kind="stable")   # hardest first
        perms.append(perm)
        p = pos[sl][perm]    # [mpc, 128, 3]
        nn2 = n2[sl][perm]
        sgl = sig[sl][perm]
        shl = sh[sl][perm]   # [mpc, 128]
        el = e[sl][perm]

        F = (f32(S0) / (shl * shl)).astype(f32)       # S0/s^2  [mpc, n]
        # per-molecule scaled lhs rows [5, n] and rhs rows [5, n]
        lhs5 = np.stack([-2.0 * p[:, :, 0] * F, -2.0 * p[:, :, 1] * F,
                         -2.0 * p[:, :, 2] * F, (nn2 + EPS_D2) * F, F],
                        axis=1).astype(f32)            # [mpc, 5, n]
        rhs5 = np.stack([p[:, :, 0] * F, p[:, :, 1] * F, p[:, :, 2] * F,
                         F, nn2 * F], axis=1).astype(f32)

        lhsp = np.zeros((10, npair, N_ATOM), dtype=f32)
        lhsp[0:5] = lhs5[0::2].transpose(1, 0, 2)
        lhsp[5:10] = lhs5[1::2].transpose(1, 0, 2)
        rhsp = np.zeros((10, npair, 2 * N_ATOM), dtype=f32)
        rhsp[0:5, :, :N_ATOM] = rhs5[0::2].transpose(1, 0, 2)
        rhsp[5:10, :, N_ATOM:] = rhs5[1::2].transpose(1, 0, 2)

        # rgs = s_i s_j / (S0 * sqrt(2 sig_i + 2 sig_j)), diag 0
        gam2 = 2.0 * (sgl[:, :, None] + sgl[:, None, :])
        rgsp = (np.einsum("mi,mj->mij", shl, shl)
                / (f32(S0) * np.sqrt(gam2))).astype(f32)
        ii = np.arange(N_ATOM)
        rgsp[:, ii, ii] = 0.0
        rgsp = np.ascontiguousarray(
            rgsp.transpose(1, 0, 2).reshape(N_ATOM, mpc * N_ATOM))

        btpk = np.empty((N_ATOM, 2 * mpc), dtype=f32)
        btpk[:, 0::2] = (el * shl / f32(S0)).T
        btpk[:, 1::2] = (shl / f32(S0)).T
        shSp = np.empty((N_ATOM, 2 * mpc), dtype=f32)
        shSp[:, 0::2] = shl.T
        shSp[:, 1::2] = shl.T
        qp = np.ascontiguousarray(Q[sl][perm]).reshape(1, mpc)
        in_maps.append({
            "lhsr_pack": np.ascontiguousarray(
                lhsp.reshape(10, npair * N_ATOM)),
            "rhsr_pack": np.ascontiguousarray(
                rhsp.reshape(10, npair * 2 * N_ATOM)),
            "rgs_pack": rgsp,
            "bt_pack": btpk,
            "bt1_pack": to_fp16(btpk * f32(RB1)),
            "bt2_pack": to_fp16(btpk * f32(RB2)),
            "shS_pack": shSp,
            "q_pack": qp,
        })
    return in_maps, perms


def run_device(in_maps, trace=False, **kw):
    if "nc" not in _CACHE:
        _CACHE["nc"] = _build_bass()
    nc = _CACHE["nc"]
    return run_bass_kernel_spmd(nc, in_maps, list(range(N_CORES)),
                                trace=trace, **kw)


def kernel(eneg, positions, node_attrs, hardness, total_charge, batch,
           atomic_numbers):
    in_maps, perms = _host_pack(eneg, positions, node_attrs, hardness,
                                total_charge, atomic_numbers)
    res = run_device(in_maps)
    outs = []
    for c in range(N_CORES):
        o = np.ascontiguousarray(res.results[c]["out"].T)   # [mol, atom]
        restored = np.empty_like(o)
        restored[perms[c]] = o
        outs.append(restored)
    full = np.concatenate(outs, axis=0).reshape(-1).astype(np.float32)
    return full
